# revision 1
# baseline (speedup 1.0000x reference)
"""GQA kernel for Trainium2, tensor-parallel over 8 NeuronCores.

Problem: B=2, S=2048, DIM=2048, 32 q-heads, 8 kv-heads, head_dim=64.
Sharding: core i owns kv-head i and q-heads 4i..4i+3 (Wq/Wk/Wv output-dim
sharded, Wo input-dim sharded). Each core computes a full [B,S,DIM] partial
of the output; the host sums the 8 partials.

Per-core dataflow (all matmul operands bf16, fp32 PSUM accumulation):
  xT (host-pretransposed, [DIM, B*S]) --> QT/KT/VT projections with head-dim
  on partitions (no on-chip transposes needed for scores);
  scores computed transposed (S^T[k,q] = KT_blk^T @ QT), exp on ScalarE with
  fused 1/sqrt(hd) scale (max-subtraction skipped: scores are N(0,1)-bounded);
  AV uses lhsT=[V | 1] so the softmax denominator lands in PSUM row 64;
  normalization via reciprocal + rank-1 broadcast matmul; O-proj consumes the
  attention output directly in its [dq, tok] layout.
"""
import sys

import numpy as np

sys.path.insert(0, "/opt/trn_rl_repo")

import ml_dtypes
import concourse.bacc as bacc
import concourse.tile as tile
from concourse import mybir
from concourse.masks import make_identity
from concourse import bass_utils

F32 = mybir.dt.float32
BF16 = mybir.dt.bfloat16

B, S, DIM = 2, 2048, 2048
N_HEADS, N_KV = 32, 8
HD = DIM // N_HEADS          # 64
G = N_HEADS // N_KV          # 4 q-heads per kv head (= per core)
DQ = G * HD                  # 256 q-proj cols per core
NCORES = 8
TOKS = B * S                 # 4096
CT = DIM // 128              # 16 contraction tiles
TT = S // 512                # 4 tok-tiles of 512 per batch
KT_N = S // 128              # 16 key tiles of 128 per batch
SM_SCALE = HD ** -0.5

_CACHE = {}


def _build():
    nc = bacc.Bacc("TRN2", debug=False, num_devices=NCORES)

    xT = nc.dram_tensor("xT", [DIM, TOKS], BF16, kind="ExternalInput")
    wq = nc.dram_tensor("wq", [DIM, DQ], BF16, kind="ExternalInput")
    wkv = nc.dram_tensor("wkv", [DIM, 2 * HD], BF16, kind="ExternalInput")
    wo = nc.dram_tensor("wo", [DQ, DIM], BF16, kind="ExternalInput")
    out_p = nc.dram_tensor("out_p", [B, S, DIM], BF16, kind="ExternalOutput")

    with tile.TileContext(nc) as tc:
        with (
            tc.tile_pool(name="wpool", bufs=1) as wpool,
            tc.tile_pool(name="xpool", bufs=2) as xpool,
            tc.tile_pool(name="actp", bufs=1) as actp,
            tc.tile_pool(name="epool", bufs=3) as epool,
            tc.tile_pool(name="small", bufs=4) as small,
            tc.tile_pool(name="pps", bufs=1, space="PSUM") as pps,
        ):
            # ---- stage weights ----
            wq_sb = wpool.tile([128, CT, 2, 128], BF16)
            nc.scalar.dma_start(
                wq_sb[:], wq.ap().rearrange("(ct p) (dt m) -> p ct dt m", p=128, m=128)
            )
            wkv_sb = wpool.tile([128, CT, 128], BF16)
            nc.scalar.dma_start(
                wkv_sb[:], wkv.ap().rearrange("(ct p) d -> p ct d", p=128)
            )
            wo_sb = wpool.tile([128, 2, 4, 512], BF16)
            nc.scalar.dma_start(
                wo_sb[:], wo.ap().rearrange("(dt p) (nt n) -> p dt nt n", p=128, n=512)
            )
            ident = wpool.tile([64, 64], BF16)
            make_identity(nc, ident[:])
            ones64 = wpool.tile([1, 64], BF16)
            nc.vector.memset(ones64[:], 1.0)

            for b in range(B):
                # ---- projections: QT[dq,tok], KT[dk,tok], VT[dv,tok] ----
                qt_g = [actp.tile([64, S], BF16, tag=f"qt{g}", name=f"qt{g}", bufs=2) for g in range(G)]
                kt = actp.tile([64, S], BF16, tag="kt", bufs=2)
                vt = actp.tile([64, S], BF16, tag="vt", bufs=2)
                v1 = actp.tile([128, KT_N, 65], BF16, tag="v1", bufs=2)
                ao2 = [actp.tile([128, S], BF16, tag=f"ao{d}", name=f"ao{d}", bufs=2) for d in range(2)]

                for tt in range(TT):
                    xc = xpool.tile([128, CT, 512], BF16, tag="xc")
                    dma_eng = nc.sync if tt % 2 == 0 else nc.gpsimd
                    dma_eng.dma_start(
                        xc[:],
                        xT.ap()[:, b * S + tt * 512: b * S + (tt + 1) * 512]
                        .rearrange("(ct p) n -> p ct n", p=128),
                    )
                    psum_q = pps.tile([128, 2, 512], F32, tag="big2", bufs=2)
                    psum_kv = pps.tile([128, 512], F32, tag="one", bufs=4)
                    for ci in range(CT):
                        st, sp = ci == 0, ci == CT - 1
                        for dt in range(2):
                            nc.tensor.matmul(psum_q[:, dt, :], wq_sb[:, ci, dt, :],
                                             xc[:, ci, :], start=st, stop=sp)
                        nc.tensor.matmul(psum_kv[:], wkv_sb[:, ci, :],
                                         xc[:, ci, :], start=st, stop=sp)
                    qs_ = slice(tt * 512, (tt + 1) * 512)
                    for g in range(G):
                        nc.vector.tensor_copy(
                            qt_g[g][:, qs_],
                            psum_q[:, g // 2, :][(g % 2) * 64:(g % 2) * 64 + 64, :],
                        )
                    nc.vector.tensor_copy(kt[:, qs_], psum_kv[0:64, :])
                    nc.vector.tensor_copy(vt[:, qs_], psum_kv[64:128, :])

                # ---- V natural [tok,dv] + ones column ----
                nc.vector.memset(v1[:, :, 64:65], 1.0)
                for ki in range(KT_N):
                    p_tr = pps.tile([128, 512], BF16, tag="one", bufs=4, name="p_tr")
                    nc.tensor.transpose(p_tr[:, 0:64], vt[:, ki * 128:(ki + 1) * 128],
                                        ident[:])
                    nc.vector.tensor_copy(v1[:, ki, 0:64], p_tr[:, 0:64])

                # ---- attention per q-head, split into two q-halves ----
                for g2 in range(2 * G):
                    g, qh = g2 // 2, g2 % 2
                    av = [pps.tile([128, 512], F32, tag="one", bufs=4, name=f"av{qs}") for qs in range(2)]
                    for ki in range(KT_N):
                        st, sp = ki == 0, ki == KT_N - 1
                        ps_s = pps.tile([128, 2, 512], F32, tag="big2", bufs=2, name="ps_s")
                        for qs in range(2):
                            nc.tensor.matmul(
                                ps_s[:, qs, :],
                                kt[:, ki * 128:(ki + 1) * 128],
                                qt_g[g][:, (qh * 2 + qs) * 512:(qh * 2 + qs + 1) * 512],
                                start=True, stop=True,
                            )
                        e_sb = epool.tile([128, 1024], BF16, tag="e", bufs=6)
                        nc.scalar.activation(e_sb[:], ps_s[:],
                                             mybir.ActivationFunctionType.Exp,
                                             scale=SM_SCALE)
                        for qs in range(2):
                            nc.tensor.matmul(
                                av[qs][0:65, :], v1[:, ki, :],
                                e_sb[:, qs * 512:(qs + 1) * 512],
                                start=st, stop=sp,
                            )
                    for qs2 in range(2):
                        qt = qh * 2 + qs2
                        raw = small.tile([65, 512], F32, tag="raw", bufs=2)
                        nc.vector.tensor_copy(raw[:], av[qs2][0:65, :])
                        den = small.tile([1, 512], F32, tag="den")
                        nc.vector.tensor_copy(den[:], raw[64:65, :])
                        nc.vector.reciprocal(den[:], den[:])
                        den_b = small.tile([1, 512], BF16, tag="denb")
                        nc.vector.tensor_copy(den_b[:], den[:])
                        p_bc = pps.tile([128, 512], F32, tag="one", bufs=4, name="p_bc")
                        nc.tensor.matmul(p_bc[0:64, :], ones64[:], den_b[:],
                                         start=True, stop=True)
                        bc_sb = small.tile([64, 512], F32, tag="bc")
                        nc.vector.tensor_copy(bc_sb[:], p_bc[0:64, :])
                        nc.vector.tensor_mul(
                            ao2[g // 2][(g % 2) * 64:(g % 2) * 64 + 64,
                                        qt * 512:(qt + 1) * 512],
                            raw[0:64, :], bc_sb[:],
                        )

                # ---- O-projection ----
                for t2 in range(S // 128):
                    o_sb = epool.tile([128, 4, 512], BF16, tag="osb", bufs=3)
                    for half in range(2):
                        po = pps.tile([128, 2, 512], F32, tag="big2", bufs=2,
                                      name="po")
                        for dt in range(2):
                            for nt in range(2):
                                nc.tensor.matmul(
                                    po[:, nt, :],
                                    ao2[dt][:, t2 * 128:(t2 + 1) * 128],
                                    wo_sb[:, dt, half * 2 + nt, :],
                                    start=dt == 0, stop=dt == 1,
                                )
                        nc.vector.tensor_copy(
                            o_sb[:, half * 2:(half + 1) * 2, :], po[:])
                    out_eng = (nc.sync, nc.gpsimd, nc.scalar)[t2 % 3]
                    out_eng.dma_start(
                        out_p.ap()[b, t2 * 128:(t2 + 1) * 128, :], o_sb[:]
                    )

    nc.compile()
    return nc


def _get_nc():
    if "nc" not in _CACHE:
        _CACHE["nc"] = _build()
    return _CACHE["nc"]


def kernel(x, Wq, Wk, Wv, Wo, _trace=False):
    nc = _get_nc()
    bf = ml_dtypes.bfloat16
    xT = np.ascontiguousarray(
        np.asarray(x, np.float32).transpose(2, 0, 1).reshape(DIM, TOKS)
    ).astype(bf)
    Wq = np.asarray(Wq, np.float32)
    Wk = np.asarray(Wk, np.float32)
    Wv = np.asarray(Wv, np.float32)
    Wo = np.asarray(Wo, np.float32)

    in_maps = []
    for c in range(NCORES):
        wq_c = Wq[:, c * DQ:(c + 1) * DQ].astype(bf)
        wkv_c = np.concatenate(
            [Wk[:, c * HD:(c + 1) * HD], Wv[:, c * HD:(c + 1) * HD]], axis=1
        ).astype(bf)
        wo_c = Wo[c * DQ:(c + 1) * DQ, :].astype(bf)
        in_maps.append({"xT": xT, "wq": np.ascontiguousarray(wq_c),
                        "wkv": np.ascontiguousarray(wkv_c),
                        "wo": np.ascontiguousarray(wo_c)})

    res = bass_utils.run_bass_kernel_spmd(
        nc, in_maps, core_ids=list(range(NCORES)), trace=_trace
    )
    out = res.results[0]["out_p"].astype(np.float64)
    for c in range(1, NCORES):
        out += res.results[c]["out_p"].astype(np.float64)
    if _trace:
        kernel.last_exec_time_ns = res.exec_time_ns
        kernel.last_results = res
    return out.astype(np.float32)


kernel.last_exec_time_ns = None



# revision 10
# speedup vs baseline: 1.5169x; 1.5169x over previous
"""GQA kernel for Trainium2, tensor-parallel over 8 NeuronCores.

Problem: B=2, S=2048, DIM=2048, 32 q-heads, 8 kv-heads, head_dim=64.
Sharding: core i owns kv-head i and q-heads 4i..4i+3 (Wq/Wk/Wv output-dim
sharded, Wo input-dim sharded). Each core computes a full [B,S,DIM] partial
of the output; the host sums the 8 partials.

v2 layout (vs v1): the kernel is ACT(exp)-bound, so everything else is
arranged to hide under the 256 exp instructions:
  - scores matmuls packed 2-at-a-time with row tiling (K=64 halves the
    array; kt holds even key-blocks on partitions 0-63 and odd blocks on
    64-127, q duplicated across both halves);
  - softmax normalization fully off the PE: DVE reciprocal_approx_fast on
    the matmul-accumulated denominator row, GpSimd partition_broadcast,
    DVE multiply reading AV PSUM directly;
  - batch 1 projections + V transposes run as PE filler work inside
    batch 0's attention; batch 0's O-projection runs inside batch 1's
    attention; only batch 1's O-projection is a tail.
"""
import sys

import numpy as np

sys.path.insert(0, "/opt/trn_rl_repo")

import ml_dtypes
import concourse.bacc as bacc
import concourse.tile as tile
from concourse import mybir
from concourse.masks import make_identity
from concourse import bass_utils

F32 = mybir.dt.float32
BF16 = mybir.dt.bfloat16

B, S, DIM = 2, 2048, 2048
N_HEADS, N_KV = 32, 8
HD = DIM // N_HEADS          # 64
G = N_HEADS // N_KV          # 4 q-heads per kv head (= per core)
DQ = G * HD                  # 256 q-proj cols per core
NCORES = 8
TOKS = B * S                 # 4096
CT = DIM // 128              # 16 contraction tiles
TT = S // 512                # 4 tok-tiles of 512 per batch
KT_N = S // 128              # 16 key blocks of 128 per batch
KP = KT_N // 2               # 8 key-block pairs
SM_SCALE = HD ** -0.5

_CACHE = {}


def _build():
    nc = bacc.Bacc("TRN2", debug=False, num_devices=NCORES)

    xT = nc.dram_tensor("xT", [DIM, TOKS], BF16, kind="ExternalInput")
    wq = nc.dram_tensor("wq", [DIM, DQ], BF16, kind="ExternalInput")
    wkv = nc.dram_tensor("wkv", [DIM, 2 * HD], BF16, kind="ExternalInput")
    wo = nc.dram_tensor("wo", [DQ, DIM], BF16, kind="ExternalInput")
    out_p = nc.dram_tensor("out_p", [B, S, DIM], BF16, kind="ExternalOutput")

    with tile.TileContext(nc) as tc:
        with (
            tc.tile_pool(name="wpool", bufs=1) as wpool,
            tc.tile_pool(name="xpool", bufs=3) as xpool,
            tc.tile_pool(name="actp", bufs=1) as actp,
            tc.tile_pool(name="epool", bufs=3) as epool,
            tc.tile_pool(name="small", bufs=2) as small,
            tc.tile_pool(name="pps", bufs=1, space="PSUM") as pps,
        ):
            # ---- stage weights (spread across issue engines) ----
            wq_sb = wpool.tile([128, CT, 2, 128], BF16)
            nc.scalar.dma_start(
                wq_sb[:], wq.ap().rearrange("(ct p) (dt m) -> p ct dt m", p=128, m=128)
            )
            wkv_sb = wpool.tile([128, CT, 128], BF16)
            nc.scalar.dma_start(
                wkv_sb[:], wkv.ap().rearrange("(ct p) d -> p ct d", p=128)
            )
            wo_sb = wpool.tile([128, 2, 4, 512], BF16)
            nc.gpsimd.dma_start(
                wo_sb[:], wo.ap().rearrange("(dt p) (nt n) -> p dt nt n", p=128, n=512)
            )
            ident = wpool.tile([64, 64], BF16)
            make_identity(nc, ident[:])

            # per-batch activation tiles (bufs=2 for cross-batch overlap)
            def batch_tiles():
                qt = [actp.tile([128, S], BF16, tag=f"qt{g}", name=f"qt{g}", bufs=2)
                      for g in range(G)]
                kt2 = actp.tile([128, KP, 128], BF16, tag="kt2", bufs=2)
                vt = actp.tile([64, S], BF16, tag="vt", bufs=2)
                v1 = actp.tile([128, KT_N, 65], BF16, tag="v1", bufs=2)
                ao2 = [actp.tile([128, S], BF16, tag=f"ao{d}", name=f"ao{d}", bufs=2)
                       for d in range(2)]
                return qt, kt2, vt, v1, ao2

            def xc_load(b, tt):
                xc = xpool.tile([128, CT, 512], BF16, tag="xc")
                nc.sync.dma_start(
                    xc[:],
                    xT.ap()[:, b * S + tt * 512: b * S + (tt + 1) * 512]
                    .rearrange("(ct p) n -> p ct n", p=128),
                )
                return xc

            def proj_copies_q(tiles, psq_half, g_pair, tt):
                # psq_half [128, 512] holds q dims for heads (2*g_pair, 2*g_pair+1)
                qt = tiles[0]
                qs_ = slice(tt * 512, (tt + 1) * 512)
                for h in range(2):
                    g = 2 * g_pair + h
                    nc.vector.tensor_copy(
                        qt[g][0:64, qs_], psq_half[h * 64:h * 64 + 64, :])
                    nc.vector.tensor_copy(qt[g][64:128, qs_], qt[g][0:64, qs_])

            def proj_copies_kv(tiles, pskv, tt):
                _, kt2, vt, _, _ = tiles
                qs_ = slice(tt * 512, (tt + 1) * 512)
                # keys tt*512.. : blocks 4tt..4tt+3; even blocks -> rows 0:64,
                # odd -> rows 64:128, pair index 2tt..2tt+1
                for c in range(2):
                    pair = 2 * tt + c
                    nc.vector.tensor_copy(kt2[0:64, pair, :],
                                          pskv[0:64, 256 * c:256 * c + 128])
                    nc.vector.tensor_copy(kt2[64:128, pair, :],
                                          pskv[0:64, 256 * c + 128:256 * c + 256])
                nc.vector.tensor_copy(vt[:, qs_], pskv[64:128, :])

            def proj_tile_full(tiles, xc, tt):
                # baseline-style dense projection for the prefix batch
                psum_q = pps.tile([128, 2, 512], F32, tag="big2", bufs=2, name="psq")
                psum_kv = pps.tile([128, 512], F32, tag="one", bufs=4, name="pskv")
                for ci in range(CT):
                    st, sp = ci == 0, ci == CT - 1
                    for dt in range(2):
                        nc.tensor.matmul(psum_q[:, dt, :], wq_sb[:, ci, dt, :],
                                         xc[:, ci, :], start=st, stop=sp)
                    nc.tensor.matmul(psum_kv[:], wkv_sb[:, ci, :],
                                     xc[:, ci, :], start=st, stop=sp)
                for gp in range(2):
                    proj_copies_q(tiles, psum_q[:, gp, :], gp, tt)
                proj_copies_kv(tiles, psum_kv, tt)

            def transpose_pair(tiles, m):
                # V natural layout for key blocks m, m+1
                _, _, vt, v1, _ = tiles
                p_tr = pps.tile([128, 512], BF16, tag="one", bufs=4, name="ptr")
                for k in range(2):
                    ki = m + k
                    nc.tensor.transpose(p_tr[:, k * 64:k * 64 + 64],
                                        vt[:, ki * 128:(ki + 1) * 128], ident[:])
                for k in range(2):
                    nc.vector.tensor_copy(v1[:, m + k, 0:64],
                                          p_tr[:, k * 64:k * 64 + 64])

            def b1_proj_fillers(tiles, b):
                """Filler closures computing batch b's projections in
                single-bank passes (each filler ~8 matmuls)."""
                fillers = []
                state = {}

                def start_pass(key, tt, dt):
                    # dt 0/1: q halves; 2: kv
                    def f():
                        xc = state[("xc", tt)]
                        p = pps.tile([128, 512], F32, tag="one", bufs=4, name="pp")
                        state[key] = p
                        for ci in range(CT // 2):
                            w = (wq_sb[:, ci, dt, :] if dt < 2
                                 else wkv_sb[:, ci, :])
                            nc.tensor.matmul(p[:], w, xc[:, ci, :],
                                             start=ci == 0, stop=False)
                    return f

                def end_pass(key, tt, dt):
                    def f():
                        xc = state[("xc", tt)]
                        p = state.pop(key)
                        for ci in range(CT // 2, CT):
                            w = (wq_sb[:, ci, dt, :] if dt < 2
                                 else wkv_sb[:, ci, :])
                            nc.tensor.matmul(p[:], w, xc[:, ci, :],
                                             start=False, stop=ci == CT - 1)
                        if dt < 2:
                            proj_copies_q(tiles, p[:], dt, tt)
                        else:
                            proj_copies_kv(tiles, p, tt)
                    return f

                def load_x(tt):
                    def f():
                        state[("xc", tt)] = xc_load(b, tt)
                    return f

                fillers.append(load_x(0))
                fillers.append(load_x(1))
                for tt in range(TT):
                    for dt in range(3):
                        key = ("p", tt, dt)
                        fillers.append(start_pass(key, tt, dt))
                        fillers.append(end_pass(key, tt, dt))
                    if tt + 2 < TT:
                        fillers.append(load_x(tt + 2))
                    fillers.append(lambda tiles=tiles, m=4 * tt: transpose_pair(tiles, m))
                    fillers.append(lambda tiles=tiles, m=4 * tt + 2: transpose_pair(tiles, m))
                return fillers

            def oproj_fillers(tiles, b):
                fillers = []
                state = {}
                ao2 = tiles[4]

                def quarter(t2, q, dma_eng):
                    t2s = slice(t2 * 128, (t2 + 1) * 128)
                    po = pps.tile([128, 512], F32, tag="one", bufs=4, name="po")
                    for dt in range(2):
                        nc.tensor.matmul(po[:], ao2[dt][:, t2s],
                                         wo_sb[:, dt, q, :],
                                         start=dt == 0, stop=dt == 1)
                    if q == 0:
                        state["osb"] = epool.tile([128, 4, 512], BF16,
                                                  tag="osb", bufs=3, name="osb")
                    o_sb = state["osb"]
                    nc.vector.tensor_copy(o_sb[:, q, :], po[:])
                    if q == 3:
                        dma_eng.dma_start(out_p.ap()[b, t2s, :], o_sb[:])

                for t2 in range(S // 128):
                    eng = (nc.sync, nc.gpsimd)[t2 % 2]
                    for q in range(4):
                        fillers.append(lambda t2=t2, q=q, eng=eng:
                                       quarter(t2, q, eng))
                return fillers

            def attention(b, tiles, fillers):
                """8 head-iterations; one filler consumed per (j, qs) slot."""
                qt, kt2, _, v1, ao2 = tiles
                n_slots = 2 * G * KP * 2
                n_f = len(fillers)
                consumed = [0]

                def maybe_fill(slot):
                    want = (slot + 1) * n_f // n_slots
                    while consumed[0] < want:
                        fillers[consumed[0]]()
                        consumed[0] += 1

                slot = 0
                for g2 in range(2 * G):
                    g, qh = g2 // 2, g2 % 2
                    av = [pps.tile([128, 512], F32, tag="one", bufs=4,
                                   name=f"av{qs}") for qs in range(2)]
                    for j in range(KP):
                        st, sp = j == 0, j == KP - 1
                        for qs in range(2):
                            qcols = slice((qh * 2 + qs) * 512,
                                          (qh * 2 + qs + 1) * 512)
                            ps_s = pps.tile([128, 2, 512], F32, tag="big2",
                                            bufs=2, name="ps_s")
                            nc.tensor.matmul(ps_s[:, 0, :], kt2[0:64, j, :],
                                             qt[g][0:64, qcols],
                                             start=True, stop=True)
                            nc.tensor.matmul(ps_s[:, 1, :], kt2[64:128, j, :],
                                             qt[g][64:128, qcols],
                                             start=True, stop=True)
                            e_sb = epool.tile([128, 2, 512], BF16, tag="e",
                                              bufs=6, name="e")
                            nc.scalar.activation(e_sb[:], ps_s[:],
                                                 mybir.ActivationFunctionType.Exp,
                                                 scale=SM_SCALE)
                            nc.tensor.matmul(av[qs][0:65, :], v1[:, 2 * j, :],
                                             e_sb[:, 0, :], start=st, stop=False)
                            nc.tensor.matmul(av[qs][0:65, :], v1[:, 2 * j + 1, :],
                                             e_sb[:, 1, :], start=False, stop=sp)
                            maybe_fill(slot)
                            slot += 1
                    # normalization: fully off the PE
                    for qs in range(2):
                        qtile = qh * 2 + qs
                        den = small.tile([1, 512], F32, tag="den", bufs=2,
                                         name="den")
                        nc.vector.tensor_copy(den[:], av[qs][64:65, :])
                        recip = small.tile([1, 512], F32, tag="recip", bufs=2,
                                           name="recip")
                        nc.vector.reciprocal_approx_fast(recip[:], den[:])
                        bc = small.tile([64, 512], F32, tag="bc", bufs=2,
                                        name="bc")
                        nc.gpsimd.partition_broadcast(bc[:], recip[:])
                        nc.vector.tensor_mul(
                            ao2[g // 2][(g % 2) * 64:(g % 2) * 64 + 64,
                                        qtile * 512:(qtile + 1) * 512],
                            av[qs][0:64, :], bc[:])
                # drain remaining fillers (shouldn't be many)
                while consumed[0] < n_f:
                    fillers[consumed[0]]()
                    consumed[0] += 1

            def oproj_tail(tiles, b):
                _, _, _, _, ao2 = tiles
                for t2 in range(S // 128):
                    t2s = slice(t2 * 128, (t2 + 1) * 128)
                    o_sb = epool.tile([128, 4, 512], BF16, tag="osb", bufs=3,
                                      name="osb")
                    for half in range(2):
                        po = pps.tile([128, 2, 512], F32, tag="big2", bufs=2,
                                      name="po2")
                        for dt in range(2):
                            for nt in range(2):
                                nc.tensor.matmul(
                                    po[:, nt, :], ao2[dt][:, t2s],
                                    wo_sb[:, dt, half * 2 + nt, :],
                                    start=dt == 0, stop=dt == 1)
                        if half == 0:
                            nc.scalar.copy(o_sb[:, 0:2, :], po[:])
                        else:
                            nc.vector.tensor_copy(o_sb[:, 2:4, :], po[:])
                    eng = (nc.sync, nc.gpsimd, nc.scalar)[t2 % 3]
                    eng.dma_start(out_p.ap()[b, t2s, :], o_sb[:])

            # ================= schedule =================
            tiles0 = batch_tiles()
            # batch 0 projections (dense prefix)
            xcs0 = [xc_load(0, tt) for tt in range(2)]
            for tt in range(TT):
                if tt + 2 < TT:
                    xcs0.append(xc_load(0, tt + 2))
                proj_tile_full(tiles0, xcs0[tt], tt)
            nc.vector.memset(tiles0[3][:, :, 64:65], 1.0)  # v1 ones column
            for m in range(0, KT_N, 2):
                transpose_pair(tiles0, m)

            # batch 1 tiles + ones (written early; fillers fill the rest)
            tiles1 = batch_tiles()
            nc.vector.memset(tiles1[3][:, :, 64:65], 1.0)

            # attention b0 with b1 projection/transpose fillers
            attention(0, tiles0, b1_proj_fillers(tiles1, 1))
            # attention b1 with b0 O-projection fillers
            attention(1, tiles1, oproj_fillers(tiles0, 0))
            # b1 O-projection tail
            oproj_tail(tiles1, 1)

    nc.compile()
    return nc


def _get_nc():
    if "nc" not in _CACHE:
        _CACHE["nc"] = _build()
    return _CACHE["nc"]


def kernel(x, Wq, Wk, Wv, Wo, _trace=False):
    nc = _get_nc()
    bf = ml_dtypes.bfloat16
    xT = np.ascontiguousarray(
        np.asarray(x, np.float32).transpose(2, 0, 1).reshape(DIM, TOKS)
    ).astype(bf)
    Wq = np.asarray(Wq, np.float32)
    Wk = np.asarray(Wk, np.float32)
    Wv = np.asarray(Wv, np.float32)
    Wo = np.asarray(Wo, np.float32)

    in_maps = []
    for c in range(NCORES):
        wq_c = Wq[:, c * DQ:(c + 1) * DQ].astype(bf)
        wkv_c = np.concatenate(
            [Wk[:, c * HD:(c + 1) * HD], Wv[:, c * HD:(c + 1) * HD]], axis=1
        ).astype(bf)
        wo_c = Wo[c * DQ:(c + 1) * DQ, :].astype(bf)
        in_maps.append({"xT": xT, "wq": np.ascontiguousarray(wq_c),
                        "wkv": np.ascontiguousarray(wkv_c),
                        "wo": np.ascontiguousarray(wo_c)})

    res = bass_utils.run_bass_kernel_spmd(
        nc, in_maps, core_ids=list(range(NCORES)), trace=_trace
    )
    out = res.results[0]["out_p"].astype(np.float64)
    for c in range(1, NCORES):
        out += res.results[c]["out_p"].astype(np.float64)
    if _trace:
        kernel.last_exec_time_ns = res.exec_time_ns
        kernel.last_results = res
    return out.astype(np.float32)


kernel.last_exec_time_ns = None


# revision 15
# speedup vs baseline: 1.5524x; 1.0234x over previous
"""GQA kernel for Trainium2, tensor-parallel over 8 NeuronCores.

Problem: B=2, S=2048, DIM=2048, 32 q-heads, 8 kv-heads, head_dim=64.
Sharding: core i owns kv-head i and q-heads 4i..4i+3 (Wq/Wk/Wv output-dim
sharded, Wo input-dim sharded). Each core computes a full [B,S,DIM] partial
of the output; the host sums the 8 partials.

The kernel is ACT(exp)-bound (256 exp instructions over the full score
matrix), so everything else hides under it:
  - scores matmuls packed 2-at-a-time with row tiling (K=64 halves the
    array; kt holds even key-blocks on partitions 0-63, odd on 64-127,
    q duplicated across both halves);
  - AV accumulates [V | 1] so the softmax denominator lands in PSUM
    row 64 (fp8 DoubleRow was tried and is ~2x faster for AV but costs
    ~3e-2 rel err - attention output magnitude averages down exactly as
    fast as the quantization noise, so fp8 e/V noise survives intact);
  - softmax normalization fully off the PE: DVE reciprocal_approx_fast,
    GpSimd partition_broadcast, DVE multiply reading AV PSUM directly;
  - cross-phase interleaving: K/V projections first so attention starts
    early, remaining Q projections + batch-1 projections + V transposes
    run as PE filler work inside batch 0's attention; batch 0's O-proj
    and the first half of batch 1's O-proj fill batch 1's attention
    (head loop is query-half-major so half the tokens finish early).
"""
import sys

import numpy as np

sys.path.insert(0, "/opt/trn_rl_repo")

import ml_dtypes
import concourse.bacc as bacc
import concourse.tile as tile
from concourse import mybir
from concourse.masks import make_identity
from concourse import bass_utils

F32 = mybir.dt.float32
BF16 = mybir.dt.bfloat16
FP8 = mybir.dt.float8e4

B, S, DIM = 2, 2048, 2048
N_HEADS, N_KV = 32, 8
HD = DIM // N_HEADS          # 64
G = N_HEADS // N_KV          # 4 q-heads per kv head (= per core)
DQ = G * HD                  # 256 q-proj cols per core
NCORES = 8
TOKS = B * S                 # 4096
CT = DIM // 128              # 16 contraction tiles
TT = S // 512                # 4 tok-tiles of 512 per batch
KT_N = S // 128              # 16 key blocks of 128 per batch
KP = KT_N // 2               # 8 key-block pairs
SM_SCALE = HD ** -0.5

_CACHE = {}


def _build():
    nc = bacc.Bacc("TRN2", debug=False, num_devices=NCORES)

    xT = nc.dram_tensor("xT", [DIM, TOKS], BF16, kind="ExternalInput")
    wq = nc.dram_tensor("wq", [DIM, DQ], BF16, kind="ExternalInput")
    wkv = nc.dram_tensor("wkv", [DIM, 2 * HD], BF16, kind="ExternalInput")
    wo = nc.dram_tensor("wo", [DQ, DIM], BF16, kind="ExternalInput")
    out_p = nc.dram_tensor("out_p", [B, S, DIM], BF16, kind="ExternalOutput")

    with tile.TileContext(nc) as tc:
        with (
            tc.tile_pool(name="wpool", bufs=1) as wpool,
            tc.tile_pool(name="xpool", bufs=4) as xpool,
            tc.tile_pool(name="actp", bufs=1) as actp,
            tc.tile_pool(name="epool", bufs=3) as epool,
            tc.tile_pool(name="small", bufs=2) as small,
            tc.tile_pool(name="pps", bufs=1, space="PSUM") as pps,
        ):
            # ---- stage inputs: x tile 0 + wq first (attention-critical) ----
            def xc_load(b, tt, eng):
                xc = xpool.tile([128, CT, 512], BF16, tag="xc")
                eng.dma_start(
                    xc[:],
                    xT.ap()[:, b * S + tt * 512: b * S + (tt + 1) * 512]
                    .rearrange("(ct p) n -> p ct n", p=128),
                )
                return xc

            xcs0 = {0: xc_load(0, 0, nc.sync)}
            wq_sb = wpool.tile([128, CT, 2, 128], BF16)
            nc.scalar.dma_start(
                wq_sb[:], wq.ap().rearrange("(ct p) (dt m) -> p ct dt m", p=128, m=128)
            )
            wkv_sb = wpool.tile([128, CT, 128], BF16)
            nc.scalar.dma_start(
                wkv_sb[:], wkv.ap().rearrange("(ct p) d -> p ct d", p=128)
            )
            xcs0[1] = xc_load(0, 1, nc.sync)
            wo_sb = wpool.tile([128, 2, 4, 512], BF16)
            nc.gpsimd.dma_start(
                wo_sb[:], wo.ap().rearrange("(dt p) (nt n) -> p dt nt n", p=128, n=512)
            )
            xcs0[2] = xc_load(0, 2, nc.scalar)
            xcs0[3] = xc_load(0, 3, nc.scalar)
            ident = wpool.tile([64, 64], BF16)
            make_identity(nc, ident[:])

            def batch_tiles():
                qt = [actp.tile([128, S], BF16, tag=f"qt{g}", name=f"qt{g}", bufs=2)
                      for g in range(G)]
                kt2 = actp.tile([128, KP, 128], BF16, tag="kt2", bufs=2)
                vt = actp.tile([64, S], BF16, tag="vt", bufs=2)
                v1 = actp.tile([128, KT_N, 65], BF16, tag="v1", bufs=2)
                ao2 = [actp.tile([128, S], BF16, tag=f"ao{d}", name=f"ao{d}", bufs=2)
                       for d in range(2)]
                return qt, kt2, vt, v1, ao2

            def proj_copies_q(tiles, psq, gp, tt):
                # psq [128, 512] holds q dims for heads (2*gp, 2*gp+1)
                qt = tiles[0]
                qs_ = slice(tt * 512, (tt + 1) * 512)
                for h in range(2):
                    g = 2 * gp + h
                    nc.vector.tensor_copy(
                        qt[g][0:64, qs_], psq[h * 64:h * 64 + 64, :])
                    nc.vector.tensor_copy(qt[g][64:128, qs_], qt[g][0:64, qs_])

            def proj_copies_kv(tiles, pskv, tt):
                _, kt2, vt, _, _ = tiles
                qs_ = slice(tt * 512, (tt + 1) * 512)
                # keys tt*512..: blocks 4tt..4tt+3; even -> rows 0:64,
                # odd -> rows 64:128, pair index 2tt, 2tt+1
                for c in range(2):
                    pair = 2 * tt + c
                    nc.vector.tensor_copy(kt2[0:64, pair, :],
                                          pskv[0:64, 256 * c:256 * c + 128])
                    nc.vector.tensor_copy(kt2[64:128, pair, :],
                                          pskv[0:64, 256 * c + 128:256 * c + 256])
                nc.vector.tensor_copy(vt[:, qs_], pskv[64:128, :])

            def make_pass(tiles, xcs, tt, dt, split=None):
                """Projection pass (dt 0/1: q head-pairs, 2: kv) as one or two
                closures accumulating into a single PSUM bank."""
                box = {}

                def run(lo, hi, first, last):
                    xc = xcs[tt]
                    if first:
                        box["p"] = pps.tile([128, 512], F32, tag="one", bufs=4,
                                            name="pp")
                    p = box["p"]
                    for ci in range(lo, hi):
                        w = wq_sb[:, ci, dt, :] if dt < 2 else wkv_sb[:, ci, :]
                        nc.tensor.matmul(p[:], w, xc[:, ci, :],
                                         start=ci == 0, stop=ci == CT - 1)
                    if last:
                        if dt < 2:
                            proj_copies_q(tiles, p[:], dt, tt)
                        else:
                            proj_copies_kv(tiles, box.pop("p"), tt)

                if split is None:
                    return [lambda: run(0, CT, True, True)]
                return [lambda: run(0, split, True, False),
                        lambda: run(split, CT, False, True)]

            def transpose_pair(tiles, m):
                # V natural layout (fp8, for DoubleRow AV) for key blocks m, m+1
                _, _, vt, v1, _ = tiles
                p_tr = pps.tile([128, 512], BF16, tag="one", bufs=4, name="ptr")
                for k in range(2):
                    ki = m + k
                    nc.tensor.transpose(p_tr[:, k * 64:k * 64 + 64],
                                        vt[:, ki * 128:(ki + 1) * 128], ident[:])
                for k in range(2):
                    nc.vector.tensor_copy(v1[:, m + k, 0:64],
                                          p_tr[:, k * 64:k * 64 + 64])

            def b1_proj_fillers(tiles, b):
                fillers = []
                xcs = {}

                def load_x(tt, eng):
                    def f():
                        xcs[tt] = xc_load(b, tt, eng)
                    return f

                fillers.append(load_x(0, nc.sync))
                fillers.append(load_x(1, nc.sync))
                for tt in range(TT):
                    for dt in (2, 0, 1):   # kv first so transposes can follow
                        fillers.extend(make_pass(tiles, xcs, tt, dt, split=CT // 2))
                    if tt + 2 < TT:
                        fillers.append(load_x(tt + 2, nc.sync))
                    fillers.append(lambda m=4 * tt: transpose_pair(tiles, m))
                    fillers.append(lambda m=4 * tt + 2: transpose_pair(tiles, m))
                return fillers

            def oproj_fillers(tiles, b, t2s_list):
                fillers = []
                state = {}
                ao2 = tiles[4]

                def quarter(t2, q, dma_eng):
                    t2s = slice(t2 * 128, (t2 + 1) * 128)
                    po = pps.tile([128, 512], F32, tag="one", bufs=4, name="po")
                    for dt in range(2):
                        nc.tensor.matmul(po[:], ao2[dt][:, t2s],
                                         wo_sb[:, dt, q, :],
                                         start=dt == 0, stop=dt == 1)
                    if q == 0:
                        state["osb"] = epool.tile([128, 4, 512], BF16,
                                                  tag="osb", bufs=3, name="osb")
                    o_sb = state["osb"]
                    nc.vector.tensor_copy(o_sb[:, q, :], po[:])
                    if q == 3:
                        dma_eng.dma_start(out_p.ap()[b, t2s, :], o_sb[:])

                for t2 in t2s_list:
                    eng = (nc.sync, nc.gpsimd)[t2 % 2]
                    for q in range(4):
                        fillers.append(lambda t2=t2, q=q, eng=eng:
                                       quarter(t2, q, eng))
                return fillers

            def attention(b, tiles, fillers, eager=0):
                """8 head-iterations (query-half-major); one filler consumed
                per (j, qs) slot, the first `eager` ones at 1/slot."""
                qt, kt2, _, v1, ao2 = tiles
                n_slots = 2 * G * KP * 2
                n_f = len(fillers)
                consumed = [0]

                def maybe_fill(slot):
                    want = (slot + 1) * n_f // n_slots
                    if slot < eager:
                        want = max(want, min(slot + 1, eager, n_f))
                    while consumed[0] < want:
                        fillers[consumed[0]]()
                        consumed[0] += 1

                slot = 0
                for qh in range(2):
                    for g in range(G):
                        av = [pps.tile([128, 512], F32, tag="one", bufs=4,
                                       name=f"av{qs}") for qs in range(2)]
                        for j in range(KP):
                            st, sp = j == 0, j == KP - 1
                            for qs in range(2):
                                qcols = slice((qh * 2 + qs) * 512,
                                              (qh * 2 + qs + 1) * 512)
                                ps_s = pps.tile([128, 2, 512], F32, tag="big2",
                                                bufs=2, name="ps_s")
                                nc.tensor.matmul(ps_s[:, 0, :], kt2[0:64, j, :],
                                                 qt[g][0:64, qcols],
                                                 start=True, stop=True)
                                nc.tensor.matmul(ps_s[:, 1, :], kt2[64:128, j, :],
                                                 qt[g][64:128, qcols],
                                                 start=True, stop=True)
                                e_sb = epool.tile([128, 2, 512], BF16, tag="e",
                                                  bufs=6, name="e")
                                nc.scalar.activation(e_sb[:], ps_s[:],
                                                     mybir.ActivationFunctionType.Exp,
                                                     scale=SM_SCALE)
                                nc.tensor.matmul(av[qs][0:65, :], v1[:, 2 * j, :],
                                                 e_sb[:, 0, :], start=st, stop=False)
                                nc.tensor.matmul(av[qs][0:65, :], v1[:, 2 * j + 1, :],
                                                 e_sb[:, 1, :], start=False, stop=sp)
                                maybe_fill(slot)
                                slot += 1
                        # normalization: fully off the PE
                        for qs in range(2):
                            qtile = qh * 2 + qs
                            den = small.tile([1, 512], F32, tag="den", bufs=2,
                                             name="den")
                            nc.vector.tensor_copy(den[:], av[qs][64:65, :])
                            recip = small.tile([1, 512], F32, tag="recip",
                                               bufs=2, name="recip")
                            nc.vector.reciprocal_approx_fast(recip[:], den[:])
                            bc = small.tile([64, 512], F32, tag="bc", bufs=2,
                                            name="bc")
                            nc.gpsimd.partition_broadcast(bc[:], recip[:])
                            nc.vector.tensor_mul(
                                ao2[g // 2][(g % 2) * 64:(g % 2) * 64 + 64,
                                            qtile * 512:(qtile + 1) * 512],
                                av[qs][0:64, :], bc[:])
                while consumed[0] < n_f:
                    fillers[consumed[0]]()
                    consumed[0] += 1

            def oproj_tail(tiles, b, t2s_list):
                _, _, _, _, ao2 = tiles
                for t2 in t2s_list:
                    t2s = slice(t2 * 128, (t2 + 1) * 128)
                    o_sb = epool.tile([128, 4, 512], BF16, tag="osb", bufs=3,
                                      name="osb")
                    for half in range(2):
                        po = pps.tile([128, 2, 512], F32, tag="big2", bufs=2,
                                      name="po2")
                        for dt in range(2):
                            for nt in range(2):
                                nc.tensor.matmul(
                                    po[:, nt, :], ao2[dt][:, t2s],
                                    wo_sb[:, dt, half * 2 + nt, :],
                                    start=dt == 0, stop=dt == 1)
                        if half == 0:
                            nc.scalar.copy(o_sb[:, 0:2, :], po[:])
                        else:
                            nc.vector.tensor_copy(o_sb[:, 2:4, :], po[:])
                    eng = (nc.sync, nc.gpsimd, nc.scalar)[t2 % 3]
                    eng.dma_start(out_p.ap()[b, t2s, :], o_sb[:])

            # ================= schedule =================
            tiles0 = batch_tiles()
            # b0 prefix: kv for all tiles, transposes, q for query-half 0
            for tt in range(TT):
                make_pass(tiles0, xcs0, tt, 2)[0]()
            nc.vector.memset(tiles0[3][:, :, 64:65], 1.0)  # v1 ones column
            for m in range(0, KT_N, 2):
                transpose_pair(tiles0, m)
            for tt in range(2):
                for dt in range(2):
                    make_pass(tiles0, xcs0, tt, dt)[0]()

            tiles1 = batch_tiles()
            nc.vector.memset(tiles1[3][:, :, 64:65], 1.0)

            # q projections for query-half 1 (eager fillers), then b1 proj
            fillers0 = []
            for tt in (2, 3):
                for dt in range(2):
                    fillers0.extend(make_pass(tiles0, xcs0, tt, dt,
                                              split=CT // 2))
            fillers0 += b1_proj_fillers(tiles1, 1)
            attention(0, tiles0, fillers0, eager=8)

            # attention b1: b0 O-proj + first half of b1 O-proj as fillers
            fillers1 = oproj_fillers(tiles0, 0, list(range(16)))
            fillers1 += oproj_fillers(tiles1, 1, list(range(8)))
            attention(1, tiles1, fillers1)
            oproj_tail(tiles1, 1, list(range(8, 16)))

    nc.compile()
    return nc


def _get_nc():
    if "nc" not in _CACHE:
        _CACHE["nc"] = _build()
    return _CACHE["nc"]


def kernel(x, Wq, Wk, Wv, Wo, _trace=False):
    nc = _get_nc()
    bf = ml_dtypes.bfloat16
    xT = np.ascontiguousarray(
        np.asarray(x, np.float32).transpose(2, 0, 1).reshape(DIM, TOKS)
    ).astype(bf)
    Wq = np.asarray(Wq, np.float32)
    Wk = np.asarray(Wk, np.float32)
    Wv = np.asarray(Wv, np.float32)
    Wo = np.asarray(Wo, np.float32)

    in_maps = []
    for c in range(NCORES):
        wq_c = Wq[:, c * DQ:(c + 1) * DQ].astype(bf)
        wkv_c = np.concatenate(
            [Wk[:, c * HD:(c + 1) * HD], Wv[:, c * HD:(c + 1) * HD]], axis=1
        ).astype(bf)
        wo_c = Wo[c * DQ:(c + 1) * DQ, :].astype(bf)
        in_maps.append({"xT": xT, "wq": np.ascontiguousarray(wq_c),
                        "wkv": np.ascontiguousarray(wkv_c),
                        "wo": np.ascontiguousarray(wo_c)})

    res = bass_utils.run_bass_kernel_spmd(
        nc, in_maps, core_ids=list(range(NCORES)), trace=_trace
    )
    out = res.results[0]["out_p"].astype(np.float64)
    for c in range(1, NCORES):
        out += res.results[c]["out_p"].astype(np.float64)
    if _trace:
        kernel.last_exec_time_ns = res.exec_time_ns
        kernel.last_results = res
    return out.astype(np.float32)


kernel.last_exec_time_ns = None


# revision 20
# speedup vs baseline: 1.5706x; 1.0117x over previous
"""GQA kernel for Trainium2, tensor-parallel over 8 NeuronCores.

Problem: B=2, S=2048, DIM=2048, 32 q-heads, 8 kv-heads, head_dim=64.
Sharding: core i owns kv-head i and q-heads 4i..4i+3 (Wq/Wk/Wv output-dim
sharded, Wo input-dim sharded). Each core computes a full [B,S,DIM] partial
of the output; the host sums the 8 partials.

The kernel is ACT(exp)-bound (256 exp instructions over the full score
matrix), so everything else hides under it:
  - scores matmuls packed 2-at-a-time with row tiling (K=64 halves the
    array; kt holds even key-blocks on partitions 0-63, odd on 64-127,
    q duplicated across both halves);
  - AV accumulates [V | 1] so the softmax denominator lands in PSUM
    row 64 (fp8 DoubleRow was tried and is ~2x faster for AV but costs
    ~3e-2 rel err - attention output magnitude averages down exactly as
    fast as the quantization noise, so fp8 e/V noise survives intact);
  - softmax normalization fully off the PE: DVE reciprocal_approx_fast,
    GpSimd partition_broadcast, DVE multiply reading AV PSUM directly;
  - cross-phase interleaving: K/V projections first so attention starts
    early, remaining Q projections + batch-1 projections + V transposes
    run as PE filler work inside batch 0's attention; batch 0's O-proj
    and the first half of batch 1's O-proj fill batch 1's attention
    (head loop is query-half-major so half the tokens finish early).
"""
import sys

import numpy as np

sys.path.insert(0, "/opt/trn_rl_repo")

import ml_dtypes
import concourse.bacc as bacc
import concourse.tile as tile
from concourse import mybir
from concourse.masks import make_identity
from concourse import bass_utils

F32 = mybir.dt.float32
BF16 = mybir.dt.bfloat16
FP8 = mybir.dt.float8e4

B, S, DIM = 2, 2048, 2048
N_HEADS, N_KV = 32, 8
HD = DIM // N_HEADS          # 64
G = N_HEADS // N_KV          # 4 q-heads per kv head (= per core)
DQ = G * HD                  # 256 q-proj cols per core
NCORES = 8
TOKS = B * S                 # 4096
CT = DIM // 128              # 16 contraction tiles
TT = S // 512                # 4 tok-tiles of 512 per batch
KT_N = S // 128              # 16 key blocks of 128 per batch
KP = KT_N // 2               # 8 key-block pairs
SM_SCALE = HD ** -0.5

_CACHE = {}


def _build():
    nc = bacc.Bacc("TRN2", debug=False, num_devices=NCORES)

    xT = nc.dram_tensor("xT", [DIM, TOKS], BF16, kind="ExternalInput")
    wq = nc.dram_tensor("wq", [DIM, DQ], BF16, kind="ExternalInput")
    wkv = nc.dram_tensor("wkv", [DIM, 2 * HD], BF16, kind="ExternalInput")
    wo = nc.dram_tensor("wo", [DQ, DIM], BF16, kind="ExternalInput")
    out_p = nc.dram_tensor("out_p", [B, S, DIM], BF16, kind="ExternalOutput")

    with tile.TileContext(nc) as tc:
        with (
            tc.tile_pool(name="wpool", bufs=1) as wpool,
            tc.tile_pool(name="xpool", bufs=4) as xpool,
            tc.tile_pool(name="actp", bufs=1) as actp,
            tc.tile_pool(name="epool", bufs=3) as epool,
            tc.tile_pool(name="small", bufs=2) as small,
            tc.tile_pool(name="pps", bufs=1, space="PSUM") as pps,
        ):
            # ---- stage inputs; x tiles split across two DMA queues so the
            # first K/V projection can start ASAP ----
            def xc_load(b, tt, split=True):
                xc = xpool.tile([128, CT, 512], BF16, tag="xc")
                cs = slice(b * S + tt * 512, b * S + (tt + 1) * 512)
                if split:
                    for h, eng in enumerate((nc.sync, nc.scalar)):
                        eng.dma_start(
                            xc[:, h * 8:(h + 1) * 8, :],
                            xT.ap()[h * 1024:(h + 1) * 1024, cs]
                            .rearrange("(ct p) n -> p ct n", p=128),
                        )
                else:
                    nc.sync.dma_start(
                        xc[:],
                        xT.ap()[:, cs].rearrange("(ct p) n -> p ct n", p=128),
                    )
                return xc

            ident = wpool.tile([64, 64], BF16)
            make_identity(nc, ident[:])
            xcs0 = {0: xc_load(0, 0)}
            wkv_sb = wpool.tile([128, CT, 128], BF16)
            nc.gpsimd.dma_start(
                wkv_sb[:], wkv.ap().rearrange("(ct p) d -> p ct d", p=128)
            )
            xcs0[1] = xc_load(0, 1)
            wq_sb = wpool.tile([128, CT, 2, 128], BF16)
            nc.gpsimd.dma_start(
                wq_sb[:], wq.ap().rearrange("(ct p) (dt m) -> p ct dt m", p=128, m=128)
            )
            xcs0[2] = xc_load(0, 2)
            xcs0[3] = xc_load(0, 3)
            wo_sb = wpool.tile([128, 2, 4, 512], BF16)
            nc.gpsimd.dma_start(
                wo_sb[:], wo.ap().rearrange("(dt p) (nt n) -> p dt nt n", p=128, n=512)
            )

            def batch_tiles():
                qt = [actp.tile([128, S], BF16, tag=f"qt{g}", name=f"qt{g}", bufs=2)
                      for g in range(G)]
                kt2 = actp.tile([128, KP, 128], BF16, tag="kt2", bufs=2)
                vt = actp.tile([64, S], BF16, tag="vt", bufs=2)
                v1 = actp.tile([128, KT_N, 65], BF16, tag="v1", bufs=2)
                ao2 = [actp.tile([128, S], BF16, tag=f"ao{d}", name=f"ao{d}", bufs=2)
                       for d in range(2)]
                return qt, kt2, vt, v1, ao2

            def proj_copies_q(tiles, psq, gp, tt):
                # psq [128, 512] holds q dims for heads (2*gp, 2*gp+1)
                qt = tiles[0]
                qs_ = slice(tt * 512, (tt + 1) * 512)
                for h in range(2):
                    g = 2 * gp + h
                    nc.vector.tensor_copy(
                        qt[g][0:64, qs_], psq[h * 64:h * 64 + 64, :])
                    nc.vector.tensor_copy(qt[g][64:128, qs_], qt[g][0:64, qs_])

            def proj_copies_kv(tiles, pskv, tt):
                _, kt2, vt, _, _ = tiles
                qs_ = slice(tt * 512, (tt + 1) * 512)
                # keys tt*512..: blocks 4tt..4tt+3; even -> rows 0:64,
                # odd -> rows 64:128, pair index 2tt, 2tt+1
                for c in range(2):
                    pair = 2 * tt + c
                    nc.vector.tensor_copy(kt2[0:64, pair, :],
                                          pskv[0:64, 256 * c:256 * c + 128])
                    nc.vector.tensor_copy(kt2[64:128, pair, :],
                                          pskv[0:64, 256 * c + 128:256 * c + 256])
                nc.vector.tensor_copy(vt[:, qs_], pskv[64:128, :])

            def make_pass(tiles, xcs, tt, dt, split=None):
                """Projection pass (dt 0/1: q head-pairs, 2: kv) as one or two
                closures accumulating into a single PSUM bank."""
                box = {}

                def run(lo, hi, first, last):
                    xc = xcs[tt]
                    if first:
                        box["p"] = pps.tile([128, 512], F32, tag="one", bufs=2,
                                            name="pp")
                    p = box["p"]
                    for ci in range(lo, hi):
                        w = wq_sb[:, ci, dt, :] if dt < 2 else wkv_sb[:, ci, :]
                        nc.tensor.matmul(p[:], w, xc[:, ci, :],
                                         start=ci == 0, stop=ci == CT - 1)
                    if last:
                        if dt < 2:
                            proj_copies_q(tiles, p[:], dt, tt)
                        else:
                            proj_copies_kv(tiles, box.pop("p"), tt)

                if split is None:
                    return [lambda: run(0, CT, True, True)]
                return [lambda: run(0, split, True, False),
                        lambda: run(split, CT, False, True)]

            def transpose_pair(tiles, m):
                # V natural layout (fp8, for DoubleRow AV) for key blocks m, m+1
                _, _, vt, v1, _ = tiles
                p_tr = pps.tile([128, 512], BF16, tag="one", bufs=2, name="ptr")
                for k in range(2):
                    ki = m + k
                    nc.tensor.transpose(p_tr[:, k * 64:k * 64 + 64],
                                        vt[:, ki * 128:(ki + 1) * 128], ident[:])
                for k in range(2):
                    nc.vector.tensor_copy(v1[:, m + k, 0:64],
                                          p_tr[:, k * 64:k * 64 + 64])

            def b1_proj_fillers(tiles, b):
                fillers = []
                xcs = {}

                def load_x(tt):
                    def f():
                        xcs[tt] = xc_load(b, tt, split=False)
                    return f

                fillers.append(load_x(0))
                fillers.append(load_x(1))
                for tt in range(TT):
                    for dt in (2, 0, 1):   # kv first so transposes can follow
                        fillers.extend(make_pass(tiles, xcs, tt, dt, split=CT // 2))
                    if tt + 2 < TT:
                        fillers.append(load_x(tt + 2))
                    fillers.append(lambda m=4 * tt: transpose_pair(tiles, m))
                    fillers.append(lambda m=4 * tt + 2: transpose_pair(tiles, m))
                return fillers

            def oproj_fillers(tiles, b, t2s_list):
                fillers = []
                state = {}
                ao2 = tiles[4]

                def quarter(t2, q, dma_eng):
                    t2s = slice(t2 * 128, (t2 + 1) * 128)
                    po = pps.tile([128, 512], F32, tag="one", bufs=2, name="po")
                    for dt in range(2):
                        nc.tensor.matmul(po[:], ao2[dt][:, t2s],
                                         wo_sb[:, dt, q, :],
                                         start=dt == 0, stop=dt == 1)
                    if q == 0:
                        state["osb"] = epool.tile([128, 4, 512], BF16,
                                                  tag="osb", bufs=3, name="osb")
                    o_sb = state["osb"]
                    nc.vector.tensor_copy(o_sb[:, q, :], po[:])
                    if q == 3:
                        dma_eng.dma_start(out_p.ap()[b, t2s, :], o_sb[:])

                for t2 in t2s_list:
                    eng = (nc.sync, nc.gpsimd)[t2 % 2]
                    for q in range(4):
                        fillers.append(lambda t2=t2, q=q, eng=eng:
                                       quarter(t2, q, eng))
                return fillers

            def attention(b, tiles, fillers, pinned=()):
                """Flat (qh, g, j, qs) step pipeline, scores emitted one step
                ahead of AV so the exp stream never stalls at head
                boundaries. `pinned` maps scores-step -> filler closures that
                MUST run at that step (dependencies of later AV steps);
                `fillers` are paced proportionally after the AV of each step.
                """
                qt, kt2, _, v1, ao2 = tiles
                steps = [(qh, g, j, qs) for qh in range(2) for g in range(G)
                         for j in range(KP) for qs in range(2)]
                n = len(steps)
                n_f = len(fillers)
                consumed = [0]
                avs = {}
                e_of = {}

                def emit_scores_exp(i):
                    qh, g, j, qs = steps[i]
                    g2 = qh * G + g
                    if j == 0 and qs == 0:
                        avs[g2] = [pps.tile([128, 512], F32, tag="av", bufs=2,
                                            name=f"av{q}") for q in range(2)]
                    qcols = slice((qh * 2 + qs) * 512, (qh * 2 + qs + 1) * 512)
                    ps_s = pps.tile([128, 2, 512], F32, tag="big2",
                                    bufs=2, name="ps_s")
                    nc.tensor.matmul(ps_s[:, 0, :], kt2[0:64, j, :],
                                     qt[g][0:64, qcols], start=True, stop=True)
                    nc.tensor.matmul(ps_s[:, 1, :], kt2[64:128, j, :],
                                     qt[g][64:128, qcols], start=True, stop=True)
                    e_sb = epool.tile([128, 2, 512], BF16, tag="e",
                                      bufs=6, name="e")
                    nc.scalar.activation(e_sb[:], ps_s[:],
                                         mybir.ActivationFunctionType.Exp,
                                         scale=SM_SCALE)
                    e_of[i] = e_sb

                def emit_av(i):
                    qh, g, j, qs = steps[i]
                    g2 = qh * G + g
                    e_sb = e_of.pop(i)
                    av = avs[g2][qs]
                    nc.tensor.matmul(av[0:65, :], v1[:, 2 * j, :],
                                     e_sb[:, 0, :], start=j == 0, stop=False)
                    nc.tensor.matmul(av[0:65, :], v1[:, 2 * j + 1, :],
                                     e_sb[:, 1, :], start=False, stop=j == KP - 1)
                    if j == KP - 1 and qs == 1:
                        norm(g2, qh, g, avs.pop(g2))

                def norm(g2, qh, g, av):
                    for qs in range(2):
                        qtile = qh * 2 + qs
                        den = small.tile([1, 512], F32, tag="den", bufs=2,
                                         name="den")
                        nc.vector.tensor_copy(den[:], av[qs][64:65, :])
                        recip = small.tile([1, 512], F32, tag="recip",
                                           bufs=2, name="recip")
                        nc.vector.reciprocal_approx_fast(recip[:], den[:])
                        bc = small.tile([64, 512], F32, tag="bc", bufs=2,
                                        name="bc")
                        nc.gpsimd.partition_broadcast(bc[:], recip[:])
                        nc.vector.tensor_mul(
                            ao2[g // 2][(g % 2) * 64:(g % 2) * 64 + 64,
                                        qtile * 512:(qtile + 1) * 512],
                            av[qs][0:64, :], bc[:])

                for i in range(n + 1):
                    if i < n:
                        emit_scores_exp(i)
                        for f in pinned.get(i, ()) if isinstance(pinned, dict) else ():
                            f()
                    if i > 0:
                        emit_av(i - 1)
                        want = i * n_f // n
                        while consumed[0] < want:
                            fillers[consumed[0]]()
                            consumed[0] += 1
                while consumed[0] < n_f:
                    fillers[consumed[0]]()
                    consumed[0] += 1

            def oproj_tail(tiles, b, t2s_list):
                _, _, _, _, ao2 = tiles
                for t2 in t2s_list:
                    t2s = slice(t2 * 128, (t2 + 1) * 128)
                    o_sb = epool.tile([128, 4, 512], BF16, tag="osb", bufs=3,
                                      name="osb")
                    for half in range(2):
                        po = pps.tile([128, 2, 512], F32, tag="big2", bufs=2,
                                      name="po2")
                        for dt in range(2):
                            for nt in range(2):
                                nc.tensor.matmul(
                                    po[:, nt, :], ao2[dt][:, t2s],
                                    wo_sb[:, dt, half * 2 + nt, :],
                                    start=dt == 0, stop=dt == 1)
                        if half == 0:
                            nc.scalar.copy(o_sb[:, 0:2, :], po[:])
                        else:
                            nc.vector.tensor_copy(o_sb[:, 2:4, :], po[:])
                    eng = (nc.sync, nc.gpsimd, nc.scalar)[t2 % 3]
                    eng.dma_start(out_p.ap()[b, t2s, :], o_sb[:])

            # ================= schedule =================
            tiles0 = batch_tiles()
            # minimal b0 prefix: K/V + transposes for the first half of the
            # keys, Q for query-half 0; everything else becomes filler work
            nc.vector.memset(tiles0[3][:, :, 64:65], 1.0)  # v1 ones column
            make_pass(tiles0, xcs0, 0, 2)[0]()
            transpose_pair(tiles0, 0)
            transpose_pair(tiles0, 2)
            make_pass(tiles0, xcs0, 1, 2)[0]()
            transpose_pair(tiles0, 4)
            transpose_pair(tiles0, 6)
            for tt in range(2):
                for dt in range(2):
                    make_pass(tiles0, xcs0, tt, dt)[0]()

            tiles1 = batch_tiles()
            nc.vector.memset(tiles1[3][:, :, 64:65], 1.0)

            # pinned early fillers: rest of b0 K/V + transposes (needed by
            # AV steps j>=4), pinned well before their consumers
            kv2 = make_pass(tiles0, xcs0, 2, 2, split=CT // 2)
            kv3 = make_pass(tiles0, xcs0, 3, 2, split=CT // 2)
            pinned0 = {
                0: [kv2[0]], 1: [kv2[1]],
                2: [lambda: transpose_pair(tiles0, 8)],
                3: [lambda: transpose_pair(tiles0, 10)],
                4: [kv3[0]], 5: [kv3[1]],
                6: [lambda: transpose_pair(tiles0, 12)],
                7: [lambda: transpose_pair(tiles0, 14)],
            }
            # paced fillers: b0 q-projections for query-half 1, then b1 proj
            fillers0 = []
            for tt in (2, 3):
                for dt in range(2):
                    fillers0.extend(make_pass(tiles0, xcs0, tt, dt,
                                              split=CT // 2))
            fillers0 += b1_proj_fillers(tiles1, 1)
            attention(0, tiles0, fillers0, pinned=pinned0)

            # attention b1: b0 O-proj + first half of b1 O-proj as fillers
            fillers1 = oproj_fillers(tiles0, 0, list(range(16)))
            fillers1 += oproj_fillers(tiles1, 1, list(range(8)))
            attention(1, tiles1, fillers1)
            oproj_tail(tiles1, 1, list(range(8, 16)))

    nc.compile()
    return nc


def _get_nc():
    if "nc" not in _CACHE:
        _CACHE["nc"] = _build()
    return _CACHE["nc"]


def kernel(x, Wq, Wk, Wv, Wo, _trace=False):
    nc = _get_nc()
    bf = ml_dtypes.bfloat16
    xT = np.ascontiguousarray(
        np.asarray(x, np.float32).transpose(2, 0, 1).reshape(DIM, TOKS)
    ).astype(bf)
    Wq = np.asarray(Wq, np.float32)
    Wk = np.asarray(Wk, np.float32)
    Wv = np.asarray(Wv, np.float32)
    Wo = np.asarray(Wo, np.float32)

    in_maps = []
    for c in range(NCORES):
        wq_c = Wq[:, c * DQ:(c + 1) * DQ].astype(bf)
        wkv_c = np.concatenate(
            [Wk[:, c * HD:(c + 1) * HD], Wv[:, c * HD:(c + 1) * HD]], axis=1
        ).astype(bf)
        wo_c = Wo[c * DQ:(c + 1) * DQ, :].astype(bf)
        in_maps.append({"xT": xT, "wq": np.ascontiguousarray(wq_c),
                        "wkv": np.ascontiguousarray(wkv_c),
                        "wo": np.ascontiguousarray(wo_c)})

    res = bass_utils.run_bass_kernel_spmd(
        nc, in_maps, core_ids=list(range(NCORES)), trace=_trace
    )
    out = res.results[0]["out_p"].astype(np.float64)
    for c in range(1, NCORES):
        out += res.results[c]["out_p"].astype(np.float64)
    if _trace:
        kernel.last_exec_time_ns = res.exec_time_ns
        kernel.last_results = res
    return out.astype(np.float32)


kernel.last_exec_time_ns = None


# revision 21
# speedup vs baseline: 1.5792x; 1.0055x over previous
"""GQA kernel for Trainium2, tensor-parallel over 8 NeuronCores.

Problem: B=2, S=2048, DIM=2048, 32 q-heads, 8 kv-heads, head_dim=64.
Sharding: core i owns kv-head i and q-heads 4i..4i+3 (Wq/Wk/Wv output-dim
sharded, Wo input-dim sharded). Each core computes a full [B,S,DIM] partial
of the output; the host sums the 8 partials.

The kernel is ACT(exp)-bound (256 exp instructions over the full score
matrix), so everything else hides under it:
  - scores matmuls packed 2-at-a-time with row tiling (K=64 halves the
    array; kt holds even key-blocks on partitions 0-63, odd on 64-127,
    q duplicated across both halves);
  - AV accumulates [V | 1] so the softmax denominator lands in PSUM
    row 64 (fp8 DoubleRow was tried and is ~2x faster for AV but costs
    ~3e-2 rel err - attention output magnitude averages down exactly as
    fast as the quantization noise, so fp8 e/V noise survives intact);
  - softmax normalization fully off the PE: DVE reciprocal_approx_fast,
    GpSimd partition_broadcast, DVE multiply reading AV PSUM directly;
  - cross-phase interleaving: K/V projections first so attention starts
    early, remaining Q projections + batch-1 projections + V transposes
    run as PE filler work inside batch 0's attention; batch 0's O-proj
    and the first half of batch 1's O-proj fill batch 1's attention
    (head loop is query-half-major so half the tokens finish early).
"""
import sys

import numpy as np

sys.path.insert(0, "/opt/trn_rl_repo")

import ml_dtypes
import concourse.bacc as bacc
import concourse.tile as tile
from concourse import mybir
from concourse.masks import make_identity
from concourse import bass_utils

F32 = mybir.dt.float32
BF16 = mybir.dt.bfloat16
FP8 = mybir.dt.float8e4

B, S, DIM = 2, 2048, 2048
N_HEADS, N_KV = 32, 8
HD = DIM // N_HEADS          # 64
G = N_HEADS // N_KV          # 4 q-heads per kv head (= per core)
DQ = G * HD                  # 256 q-proj cols per core
NCORES = 8
TOKS = B * S                 # 4096
CT = DIM // 128              # 16 contraction tiles
TT = S // 512                # 4 tok-tiles of 512 per batch
KT_N = S // 128              # 16 key blocks of 128 per batch
KP = KT_N // 2               # 8 key-block pairs
SM_SCALE = HD ** -0.5

_CACHE = {}


def _build():
    nc = bacc.Bacc("TRN2", debug=False, num_devices=NCORES)

    xT = nc.dram_tensor("xT", [DIM, TOKS], BF16, kind="ExternalInput")
    wq = nc.dram_tensor("wq", [DIM, DQ], BF16, kind="ExternalInput")
    wkv = nc.dram_tensor("wkv", [DIM, 2 * HD], BF16, kind="ExternalInput")
    wo = nc.dram_tensor("wo", [DQ, DIM], BF16, kind="ExternalInput")
    out_p = nc.dram_tensor("out_p", [B, S, DIM], BF16, kind="ExternalOutput")

    with tile.TileContext(nc) as tc:
        with (
            tc.tile_pool(name="wpool", bufs=1) as wpool,
            tc.tile_pool(name="xpool", bufs=4) as xpool,
            tc.tile_pool(name="actp", bufs=1) as actp,
            tc.tile_pool(name="epool", bufs=3) as epool,
            tc.tile_pool(name="small", bufs=2) as small,
            tc.tile_pool(name="pps", bufs=1, space="PSUM") as pps,
        ):
            # ---- stage inputs; x tiles split across two DMA queues so the
            # first K/V projection can start ASAP ----
            def xc_load(b, tt, split=True):
                xc = xpool.tile([128, CT, 512], BF16, tag="xc")
                cs = slice(b * S + tt * 512, b * S + (tt + 1) * 512)
                if split:
                    for h, eng in enumerate((nc.sync, nc.scalar)):
                        eng.dma_start(
                            xc[:, h * 8:(h + 1) * 8, :],
                            xT.ap()[h * 1024:(h + 1) * 1024, cs]
                            .rearrange("(ct p) n -> p ct n", p=128),
                        )
                else:
                    nc.sync.dma_start(
                        xc[:],
                        xT.ap()[:, cs].rearrange("(ct p) n -> p ct n", p=128),
                    )
                return xc

            ident = wpool.tile([64, 64], BF16)
            make_identity(nc, ident[:])
            xcs0 = {0: xc_load(0, 0)}
            wkv_sb = wpool.tile([128, CT, 128], BF16)
            nc.gpsimd.dma_start(
                wkv_sb[:], wkv.ap().rearrange("(ct p) d -> p ct d", p=128)
            )
            xcs0[1] = xc_load(0, 1)
            wq_sb = wpool.tile([128, CT, 2, 128], BF16)
            nc.gpsimd.dma_start(
                wq_sb[:], wq.ap().rearrange("(ct p) (dt m) -> p ct dt m", p=128, m=128)
            )
            xcs0[2] = xc_load(0, 2)
            xcs0[3] = xc_load(0, 3)
            wo_sb = wpool.tile([128, 2, 4, 512], BF16)
            nc.gpsimd.dma_start(
                wo_sb[:], wo.ap().rearrange("(dt p) (nt n) -> p dt nt n", p=128, n=512)
            )

            def batch_tiles():
                qt = [actp.tile([128, S], BF16, tag=f"qt{g}", name=f"qt{g}", bufs=2)
                      for g in range(G)]
                kt2 = actp.tile([128, KP, 128], BF16, tag="kt2", bufs=2)
                vt = actp.tile([64, S], BF16, tag="vt", bufs=2)
                v1 = actp.tile([128, KT_N, 65], BF16, tag="v1", bufs=2)
                ao2 = [actp.tile([128, S], BF16, tag=f"ao{d}", name=f"ao{d}", bufs=2)
                       for d in range(2)]
                return qt, kt2, vt, v1, ao2

            def proj_copies_q(tiles, psq, gp, tt):
                # psq [128, 512] holds q dims for heads (2*gp, 2*gp+1)
                qt = tiles[0]
                qs_ = slice(tt * 512, (tt + 1) * 512)
                for h in range(2):
                    g = 2 * gp + h
                    nc.vector.tensor_copy(
                        qt[g][0:64, qs_], psq[h * 64:h * 64 + 64, :])
                    nc.vector.tensor_copy(qt[g][64:128, qs_], qt[g][0:64, qs_])

            def proj_copies_kv(tiles, pskv, tt):
                _, kt2, vt, _, _ = tiles
                qs_ = slice(tt * 512, (tt + 1) * 512)
                # keys tt*512..: blocks 4tt..4tt+3; even -> rows 0:64,
                # odd -> rows 64:128, pair index 2tt, 2tt+1
                for c in range(2):
                    pair = 2 * tt + c
                    nc.vector.tensor_copy(kt2[0:64, pair, :],
                                          pskv[0:64, 256 * c:256 * c + 128])
                    nc.vector.tensor_copy(kt2[64:128, pair, :],
                                          pskv[0:64, 256 * c + 128:256 * c + 256])
                nc.vector.tensor_copy(vt[:, qs_], pskv[64:128, :])

            def make_pass(tiles, xcs, tt, dt, split=None):
                """Projection pass (dt 0/1: q head-pairs, 2: kv) as one or two
                closures accumulating into a single PSUM bank."""
                box = {}

                def run(lo, hi, first, last):
                    xc = xcs[tt]
                    if first:
                        box["p"] = pps.tile([128, 512], F32, tag="one", bufs=2,
                                            name="pp")
                    p = box["p"]
                    for ci in range(lo, hi):
                        w = wq_sb[:, ci, dt, :] if dt < 2 else wkv_sb[:, ci, :]
                        nc.tensor.matmul(p[:], w, xc[:, ci, :],
                                         start=ci == 0, stop=ci == CT - 1)
                    if last:
                        if dt < 2:
                            proj_copies_q(tiles, p[:], dt, tt)
                        else:
                            proj_copies_kv(tiles, box.pop("p"), tt)

                if split is None:
                    return [lambda: run(0, CT, True, True)]
                return [lambda: run(0, split, True, False),
                        lambda: run(split, CT, False, True)]

            def transpose_pair(tiles, m):
                # V natural layout (fp8, for DoubleRow AV) for key blocks m, m+1
                _, _, vt, v1, _ = tiles
                p_tr = pps.tile([128, 512], BF16, tag="one", bufs=2, name="ptr")
                for k in range(2):
                    ki = m + k
                    nc.tensor.transpose(p_tr[:, k * 64:k * 64 + 64],
                                        vt[:, ki * 128:(ki + 1) * 128], ident[:])
                for k in range(2):
                    nc.vector.tensor_copy(v1[:, m + k, 0:64],
                                          p_tr[:, k * 64:k * 64 + 64])

            def b1_proj_fillers(tiles, b):
                fillers = []
                xcs = {}

                def load_x(tt):
                    def f():
                        xcs[tt] = xc_load(b, tt, split=False)
                    return f

                fillers.append(load_x(0))
                fillers.append(load_x(1))
                for tt in range(TT):
                    for dt in (2, 0, 1):   # kv first so transposes can follow
                        fillers.extend(make_pass(tiles, xcs, tt, dt, split=CT // 2))
                    if tt + 2 < TT:
                        fillers.append(load_x(tt + 2))
                    fillers.append(lambda m=4 * tt: transpose_pair(tiles, m))
                    fillers.append(lambda m=4 * tt + 2: transpose_pair(tiles, m))
                return fillers

            def oproj_fillers(tiles, b, t2s_list):
                fillers = []
                state = {}
                ao2 = tiles[4]

                def quarter(t2, q, dma_eng):
                    t2s = slice(t2 * 128, (t2 + 1) * 128)
                    po = pps.tile([128, 512], F32, tag="one", bufs=2, name="po")
                    for dt in range(2):
                        nc.tensor.matmul(po[:], ao2[dt][:, t2s],
                                         wo_sb[:, dt, q, :],
                                         start=dt == 0, stop=dt == 1)
                    if q == 0:
                        state["osb"] = epool.tile([128, 4, 512], BF16,
                                                  tag="osb", bufs=3, name="osb")
                    o_sb = state["osb"]
                    nc.vector.tensor_copy(o_sb[:, q, :], po[:])
                    if q == 3:
                        dma_eng.dma_start(out_p.ap()[b, t2s, :], o_sb[:])

                for t2 in t2s_list:
                    eng = (nc.sync, nc.gpsimd)[t2 % 2]
                    for q in range(4):
                        fillers.append(lambda t2=t2, q=q, eng=eng:
                                       quarter(t2, q, eng))
                return fillers

            def attention(b, tiles, fillers, pinned=()):
                """Flat (qh, g, j, qs) step pipeline, scores emitted one step
                ahead of AV so the exp stream never stalls at head
                boundaries. `pinned` maps scores-step -> filler closures that
                MUST run at that step (dependencies of later AV steps);
                `fillers` are paced proportionally after the AV of each step.
                """
                qt, kt2, _, v1, ao2 = tiles
                steps = [(qh, g, j, qs) for qh in range(2) for g in range(G)
                         for j in range(KP) for qs in range(2)]
                n = len(steps)
                n_f = len(fillers)
                consumed = [0]
                avs = {}
                e_of = {}

                def emit_scores_exp(i):
                    qh, g, j, qs = steps[i]
                    g2 = qh * G + g
                    if j == 0 and qs == 0:
                        avs[g2] = [pps.tile([128, 512], F32, tag="av", bufs=2,
                                            name=f"av{q}") for q in range(2)]
                    qcols = slice((qh * 2 + qs) * 512, (qh * 2 + qs + 1) * 512)
                    ps_s = pps.tile([128, 2, 512], F32, tag="big2",
                                    bufs=2, name="ps_s")
                    nc.tensor.matmul(ps_s[:, 0, :], kt2[0:64, j, :],
                                     qt[g][0:64, qcols], start=True, stop=True)
                    nc.tensor.matmul(ps_s[:, 1, :], kt2[64:128, j, :],
                                     qt[g][64:128, qcols], start=True, stop=True)
                    e_sb = epool.tile([128, 2, 512], BF16, tag="e",
                                      bufs=6, name="e")
                    nc.scalar.activation(e_sb[:], ps_s[:],
                                         mybir.ActivationFunctionType.Exp,
                                         scale=SM_SCALE)
                    e_of[i] = e_sb

                def emit_av(i):
                    qh, g, j, qs = steps[i]
                    g2 = qh * G + g
                    e_sb = e_of.pop(i)
                    av = avs[g2][qs]
                    nc.tensor.matmul(av[0:65, :], v1[:, 2 * j, :],
                                     e_sb[:, 0, :], start=j == 0, stop=False)
                    nc.tensor.matmul(av[0:65, :], v1[:, 2 * j + 1, :],
                                     e_sb[:, 1, :], start=False, stop=j == KP - 1)
                    if j == KP - 1:
                        # normalize this query-tile immediately so its av
                        # bank frees with maximum slack before reuse
                        norm(qh, g, qs, av)
                        if qs == 1:
                            avs.pop(g2)

                def norm(qh, g, qs, av):
                    qtile = qh * 2 + qs
                    den = small.tile([1, 512], F32, tag="den", bufs=2,
                                     name="den")
                    nc.vector.tensor_copy(den[:], av[64:65, :])
                    recip = small.tile([1, 512], F32, tag="recip",
                                       bufs=2, name="recip")
                    nc.vector.reciprocal_approx_fast(recip[:], den[:])
                    bc = small.tile([64, 512], F32, tag="bc", bufs=2,
                                    name="bc")
                    nc.gpsimd.partition_broadcast(bc[:], recip[:])
                    nc.vector.tensor_mul(
                        ao2[g // 2][(g % 2) * 64:(g % 2) * 64 + 64,
                                    qtile * 512:(qtile + 1) * 512],
                        av[0:64, :], bc[:])

                for i in range(n + 1):
                    if i < n:
                        emit_scores_exp(i)
                        for f in pinned.get(i, ()) if isinstance(pinned, dict) else ():
                            f()
                    if i > 0:
                        emit_av(i - 1)
                        want = i * n_f // n
                        while consumed[0] < want:
                            fillers[consumed[0]]()
                            consumed[0] += 1
                while consumed[0] < n_f:
                    fillers[consumed[0]]()
                    consumed[0] += 1

            def oproj_tail(tiles, b, t2s_list):
                _, _, _, _, ao2 = tiles
                for t2 in t2s_list:
                    t2s = slice(t2 * 128, (t2 + 1) * 128)
                    o_sb = epool.tile([128, 4, 512], BF16, tag="osb", bufs=3,
                                      name="osb")
                    for half in range(2):
                        po = pps.tile([128, 2, 512], F32, tag="big2", bufs=2,
                                      name="po2")
                        for dt in range(2):
                            for nt in range(2):
                                nc.tensor.matmul(
                                    po[:, nt, :], ao2[dt][:, t2s],
                                    wo_sb[:, dt, half * 2 + nt, :],
                                    start=dt == 0, stop=dt == 1)
                        if half == 0:
                            nc.scalar.copy(o_sb[:, 0:2, :], po[:])
                        else:
                            nc.vector.tensor_copy(o_sb[:, 2:4, :], po[:])
                    eng = (nc.sync, nc.gpsimd, nc.scalar)[t2 % 3]
                    eng.dma_start(out_p.ap()[b, t2s, :], o_sb[:])

            # ================= schedule =================
            tiles0 = batch_tiles()
            # minimal b0 prefix: K/V + transposes for the first half of the
            # keys, Q for query-half 0; everything else becomes filler work
            nc.vector.memset(tiles0[3][:, :, 64:65], 1.0)  # v1 ones column
            make_pass(tiles0, xcs0, 0, 2)[0]()
            transpose_pair(tiles0, 0)
            transpose_pair(tiles0, 2)
            make_pass(tiles0, xcs0, 1, 2)[0]()
            transpose_pair(tiles0, 4)
            transpose_pair(tiles0, 6)
            for tt in range(2):
                for dt in range(2):
                    make_pass(tiles0, xcs0, tt, dt)[0]()

            tiles1 = batch_tiles()
            nc.vector.memset(tiles1[3][:, :, 64:65], 1.0)

            # pinned early fillers: rest of b0 K/V + transposes (needed by
            # AV steps j>=4), pinned well before their consumers
            kv2 = make_pass(tiles0, xcs0, 2, 2, split=CT // 2)
            kv3 = make_pass(tiles0, xcs0, 3, 2, split=CT // 2)
            pinned0 = {
                0: [kv2[0]], 1: [kv2[1]],
                2: [lambda: transpose_pair(tiles0, 8)],
                3: [lambda: transpose_pair(tiles0, 10)],
                4: [kv3[0]], 5: [kv3[1]],
                6: [lambda: transpose_pair(tiles0, 12)],
                7: [lambda: transpose_pair(tiles0, 14)],
            }
            # paced fillers: b0 q-projections for query-half 1, then b1 proj
            fillers0 = []
            for tt in (2, 3):
                for dt in range(2):
                    fillers0.extend(make_pass(tiles0, xcs0, tt, dt,
                                              split=CT // 2))
            fillers0 += b1_proj_fillers(tiles1, 1)
            attention(0, tiles0, fillers0, pinned=pinned0)

            # attention b1: b0 O-proj + first half of b1 O-proj as fillers
            fillers1 = oproj_fillers(tiles0, 0, list(range(16)))
            fillers1 += oproj_fillers(tiles1, 1, list(range(8)))
            attention(1, tiles1, fillers1)
            oproj_tail(tiles1, 1, list(range(8, 16)))

    nc.compile()
    return nc


def _get_nc():
    if "nc" not in _CACHE:
        _CACHE["nc"] = _build()
    return _CACHE["nc"]


def kernel(x, Wq, Wk, Wv, Wo, _trace=False):
    nc = _get_nc()
    bf = ml_dtypes.bfloat16
    xT = np.ascontiguousarray(
        np.asarray(x, np.float32).transpose(2, 0, 1).reshape(DIM, TOKS)
    ).astype(bf)
    Wq = np.asarray(Wq, np.float32)
    Wk = np.asarray(Wk, np.float32)
    Wv = np.asarray(Wv, np.float32)
    Wo = np.asarray(Wo, np.float32)

    in_maps = []
    for c in range(NCORES):
        wq_c = Wq[:, c * DQ:(c + 1) * DQ].astype(bf)
        wkv_c = np.concatenate(
            [Wk[:, c * HD:(c + 1) * HD], Wv[:, c * HD:(c + 1) * HD]], axis=1
        ).astype(bf)
        wo_c = Wo[c * DQ:(c + 1) * DQ, :].astype(bf)
        in_maps.append({"xT": xT, "wq": np.ascontiguousarray(wq_c),
                        "wkv": np.ascontiguousarray(wkv_c),
                        "wo": np.ascontiguousarray(wo_c)})

    res = bass_utils.run_bass_kernel_spmd(
        nc, in_maps, core_ids=list(range(NCORES)), trace=_trace
    )
    out = res.results[0]["out_p"].astype(np.float64)
    for c in range(1, NCORES):
        out += res.results[c]["out_p"].astype(np.float64)
    if _trace:
        kernel.last_exec_time_ns = res.exec_time_ns
        kernel.last_results = res
    return out.astype(np.float32)


kernel.last_exec_time_ns = None


# revision 22
# speedup vs baseline: 1.5800x; 1.0005x over previous
"""GQA kernel for Trainium2, tensor-parallel over 8 NeuronCores.

Problem: B=2, S=2048, DIM=2048, 32 q-heads, 8 kv-heads, head_dim=64.
Sharding: core i owns kv-head i and q-heads 4i..4i+3 (Wq/Wk/Wv output-dim
sharded, Wo input-dim sharded). Each core computes a full [B,S,DIM] partial
of the output; the host sums the 8 partials.

The kernel is ACT(exp)-bound (256 exp instructions over the full score
matrix), so everything else hides under it:
  - scores matmuls packed 2-at-a-time with row tiling (K=64 halves the
    array; kt holds even key-blocks on partitions 0-63, odd on 64-127,
    q duplicated across both halves);
  - AV accumulates [V | 1] so the softmax denominator lands in PSUM
    row 64 (fp8 DoubleRow was tried and is ~2x faster for AV but costs
    ~3e-2 rel err - attention output magnitude averages down exactly as
    fast as the quantization noise, so fp8 e/V noise survives intact);
  - softmax normalization fully off the PE: DVE reciprocal_approx_fast,
    GpSimd partition_broadcast, DVE multiply reading AV PSUM directly;
  - cross-phase interleaving: K/V projections first so attention starts
    early, remaining Q projections + batch-1 projections + V transposes
    run as PE filler work inside batch 0's attention; batch 0's O-proj
    and the first half of batch 1's O-proj fill batch 1's attention
    (head loop is query-half-major so half the tokens finish early).
"""
import sys

import numpy as np

sys.path.insert(0, "/opt/trn_rl_repo")

import ml_dtypes
import concourse.bacc as bacc
import concourse.tile as tile
from concourse import mybir
from concourse.masks import make_identity
from concourse import bass_utils

F32 = mybir.dt.float32
BF16 = mybir.dt.bfloat16
FP8 = mybir.dt.float8e4

B, S, DIM = 2, 2048, 2048
N_HEADS, N_KV = 32, 8
HD = DIM // N_HEADS          # 64
G = N_HEADS // N_KV          # 4 q-heads per kv head (= per core)
DQ = G * HD                  # 256 q-proj cols per core
NCORES = 8
TOKS = B * S                 # 4096
CT = DIM // 128              # 16 contraction tiles
TT = S // 512                # 4 tok-tiles of 512 per batch
KT_N = S // 128              # 16 key blocks of 128 per batch
KP = KT_N // 2               # 8 key-block pairs
SM_SCALE = HD ** -0.5

_CACHE = {}


def _build():
    nc = bacc.Bacc("TRN2", debug=False, num_devices=NCORES)

    xT = nc.dram_tensor("xT", [DIM, TOKS], BF16, kind="ExternalInput")
    wq = nc.dram_tensor("wq", [DIM, DQ], BF16, kind="ExternalInput")
    wkv = nc.dram_tensor("wkv", [DIM, 2 * HD], BF16, kind="ExternalInput")
    wo = nc.dram_tensor("wo", [DQ, DIM], BF16, kind="ExternalInput")
    out_p = nc.dram_tensor("out_p", [B, S, DIM], BF16, kind="ExternalOutput")

    with tile.TileContext(nc) as tc:
        with (
            tc.tile_pool(name="wpool", bufs=1) as wpool,
            tc.tile_pool(name="xpool", bufs=4) as xpool,
            tc.tile_pool(name="actp", bufs=1) as actp,
            tc.tile_pool(name="epool", bufs=3) as epool,
            tc.tile_pool(name="small", bufs=2) as small,
            tc.tile_pool(name="pps", bufs=1, space="PSUM") as pps,
        ):
            # ---- stage inputs; x tiles split across two DMA queues so the
            # first K/V projection can start ASAP ----
            def xc_load(b, tt, split=True):
                xc = xpool.tile([128, CT, 512], BF16, tag="xc")
                cs = slice(b * S + tt * 512, b * S + (tt + 1) * 512)
                if split:
                    for h, eng in enumerate((nc.sync, nc.scalar)):
                        eng.dma_start(
                            xc[:, h * 8:(h + 1) * 8, :],
                            xT.ap()[h * 1024:(h + 1) * 1024, cs]
                            .rearrange("(ct p) n -> p ct n", p=128),
                        )
                else:
                    nc.sync.dma_start(
                        xc[:],
                        xT.ap()[:, cs].rearrange("(ct p) n -> p ct n", p=128),
                    )
                return xc

            ident = wpool.tile([64, 64], BF16)
            make_identity(nc, ident[:])
            xcs0 = {0: xc_load(0, 0)}
            wkv_sb = wpool.tile([128, CT, 128], BF16)
            nc.gpsimd.dma_start(
                wkv_sb[:], wkv.ap().rearrange("(ct p) d -> p ct d", p=128)
            )
            xcs0[1] = xc_load(0, 1)
            wq_sb = wpool.tile([128, CT, 2, 128], BF16)
            nc.gpsimd.dma_start(
                wq_sb[:], wq.ap().rearrange("(ct p) (dt m) -> p ct dt m", p=128, m=128)
            )
            xcs0[2] = xc_load(0, 2)
            xcs0[3] = xc_load(0, 3)
            wo_sb = wpool.tile([128, 2, 4, 512], BF16)
            nc.gpsimd.dma_start(
                wo_sb[:], wo.ap().rearrange("(dt p) (nt n) -> p dt nt n", p=128, n=512)
            )

            def batch_tiles():
                qt = [actp.tile([128, S], BF16, tag=f"qt{g}", name=f"qt{g}", bufs=2)
                      for g in range(G)]
                kt2 = actp.tile([128, KP, 128], BF16, tag="kt2", bufs=2)
                vt = actp.tile([64, S], BF16, tag="vt", bufs=2)
                v1 = actp.tile([128, KT_N, 65], BF16, tag="v1", bufs=2)
                ao2 = [actp.tile([128, S], BF16, tag=f"ao{d}", name=f"ao{d}", bufs=2)
                       for d in range(2)]
                return qt, kt2, vt, v1, ao2

            def proj_copies_q(tiles, psq, gp, tt):
                # psq [128, 512] holds q dims for heads (2*gp, 2*gp+1)
                qt = tiles[0]
                qs_ = slice(tt * 512, (tt + 1) * 512)
                for h in range(2):
                    g = 2 * gp + h
                    nc.vector.tensor_copy(
                        qt[g][0:64, qs_], psq[h * 64:h * 64 + 64, :])
                    nc.vector.tensor_copy(qt[g][64:128, qs_], qt[g][0:64, qs_])

            def proj_copies_kv(tiles, pskv, tt):
                _, kt2, vt, _, _ = tiles
                qs_ = slice(tt * 512, (tt + 1) * 512)
                # keys tt*512..: blocks 4tt..4tt+3; even -> rows 0:64,
                # odd -> rows 64:128, pair index 2tt, 2tt+1
                for c in range(2):
                    pair = 2 * tt + c
                    nc.vector.tensor_copy(kt2[0:64, pair, :],
                                          pskv[0:64, 256 * c:256 * c + 128])
                    nc.vector.tensor_copy(kt2[64:128, pair, :],
                                          pskv[0:64, 256 * c + 128:256 * c + 256])
                nc.vector.tensor_copy(vt[:, qs_], pskv[64:128, :])

            def make_pass(tiles, xcs, tt, dt, split=None):
                """Projection pass (dt 0/1: q head-pairs, 2: kv) as one or two
                closures accumulating into a single PSUM bank."""
                box = {}

                def run(lo, hi, first, last):
                    xc = xcs[tt]
                    if first:
                        box["p"] = pps.tile([128, 512], F32, tag="one", bufs=2,
                                            name="pp")
                    p = box["p"]
                    for ci in range(lo, hi):
                        w = wq_sb[:, ci, dt, :] if dt < 2 else wkv_sb[:, ci, :]
                        nc.tensor.matmul(p[:], w, xc[:, ci, :],
                                         start=ci == 0, stop=ci == CT - 1)
                    if last:
                        if dt < 2:
                            proj_copies_q(tiles, p[:], dt, tt)
                        else:
                            proj_copies_kv(tiles, box.pop("p"), tt)

                if split is None:
                    return [lambda: run(0, CT, True, True)]
                return [lambda: run(0, split, True, False),
                        lambda: run(split, CT, False, True)]

            def transpose_pair(tiles, m):
                # V natural layout (fp8, for DoubleRow AV) for key blocks m, m+1
                _, _, vt, v1, _ = tiles
                p_tr = pps.tile([128, 512], BF16, tag="one", bufs=2, name="ptr")
                for k in range(2):
                    ki = m + k
                    nc.tensor.transpose(p_tr[:, k * 64:k * 64 + 64],
                                        vt[:, ki * 128:(ki + 1) * 128], ident[:])
                for k in range(2):
                    nc.vector.tensor_copy(v1[:, m + k, 0:64],
                                          p_tr[:, k * 64:k * 64 + 64])

            def b1_proj_fillers(tiles, b):
                fillers = []
                xcs = {}

                def load_x(tt):
                    def f():
                        xcs[tt] = xc_load(b, tt, split=False)
                    return f

                fillers.append(load_x(0))
                fillers.append(load_x(1))
                for tt in range(TT):
                    for dt in (2, 0, 1):   # kv first so transposes can follow
                        fillers.extend(make_pass(tiles, xcs, tt, dt, split=CT // 2))
                    if tt + 2 < TT:
                        fillers.append(load_x(tt + 2))
                    fillers.append(lambda m=4 * tt: transpose_pair(tiles, m))
                    fillers.append(lambda m=4 * tt + 2: transpose_pair(tiles, m))
                return fillers

            def oproj_fillers(tiles, b, t2s_list):
                fillers = []
                state = {}
                ao2 = tiles[4]

                def quarter(t2, q, dma_eng):
                    t2s = slice(t2 * 128, (t2 + 1) * 128)
                    po = pps.tile([128, 512], F32, tag="one", bufs=2, name="po")
                    for dt in range(2):
                        nc.tensor.matmul(po[:], ao2[dt][:, t2s],
                                         wo_sb[:, dt, q, :],
                                         start=dt == 0, stop=dt == 1)
                    if q == 0:
                        state["osb"] = epool.tile([128, 4, 512], BF16,
                                                  tag="osb", bufs=3, name="osb")
                    o_sb = state["osb"]
                    nc.vector.tensor_copy(o_sb[:, q, :], po[:])
                    if q == 3:
                        dma_eng.dma_start(out_p.ap()[b, t2s, :], o_sb[:])

                for t2 in t2s_list:
                    eng = (nc.sync, nc.gpsimd)[t2 % 2]
                    for q in range(4):
                        fillers.append(lambda t2=t2, q=q, eng=eng:
                                       quarter(t2, q, eng))
                return fillers

            def attention(b, tiles, fillers, pinned=()):
                """Flat (qh, g, j, qs) step pipeline, scores emitted one step
                ahead of AV so the exp stream never stalls at head
                boundaries. `pinned` maps scores-step -> filler closures that
                MUST run at that step (dependencies of later AV steps);
                `fillers` are paced proportionally after the AV of each step.
                """
                qt, kt2, _, v1, ao2 = tiles
                steps = [(qh, g, j, qs) for qh in range(2) for g in range(G)
                         for j in range(KP) for qs in range(2)]
                n = len(steps)
                n_f = len(fillers)
                consumed = [0]
                avs = {}
                e_of = {}

                def emit_scores_exp(i):
                    qh, g, j, qs = steps[i]
                    g2 = qh * G + g
                    if j == 0 and qs == 0:
                        avs[g2] = [pps.tile([128, 512], F32, tag="av", bufs=2,
                                            name=f"av{q}") for q in range(2)]
                    qcols = slice((qh * 2 + qs) * 512, (qh * 2 + qs + 1) * 512)
                    ps_s = pps.tile([128, 2, 512], F32, tag="big2",
                                    bufs=2, name="ps_s")
                    nc.tensor.matmul(ps_s[:, 0, :], kt2[0:64, j, :],
                                     qt[g][0:64, qcols], start=True, stop=True)
                    nc.tensor.matmul(ps_s[:, 1, :], kt2[64:128, j, :],
                                     qt[g][64:128, qcols], start=True, stop=True)
                    e_sb = epool.tile([128, 2, 512], BF16, tag="e",
                                      bufs=6, name="e")
                    nc.scalar.activation(e_sb[:], ps_s[:],
                                         mybir.ActivationFunctionType.Exp,
                                         scale=SM_SCALE)
                    e_of[i] = e_sb

                def emit_av(i):
                    qh, g, j, qs = steps[i]
                    g2 = qh * G + g
                    e_sb = e_of.pop(i)
                    av = avs[g2][qs]
                    nc.tensor.matmul(av[0:65, :], v1[:, 2 * j, :],
                                     e_sb[:, 0, :], start=j == 0, stop=False)
                    nc.tensor.matmul(av[0:65, :], v1[:, 2 * j + 1, :],
                                     e_sb[:, 1, :], start=False, stop=j == KP - 1)
                    if j == KP - 1:
                        # normalize this query-tile immediately so its av
                        # bank frees with maximum slack before reuse
                        norm(qh, g, qs, av)
                        if qs == 1:
                            avs.pop(g2)

                def norm(qh, g, qs, av):
                    qtile = qh * 2 + qs
                    den = small.tile([1, 512], F32, tag="den", bufs=2,
                                     name="den")
                    nc.vector.tensor_copy(den[:], av[64:65, :])
                    recip = small.tile([1, 512], F32, tag="recip",
                                       bufs=2, name="recip")
                    nc.vector.reciprocal_approx_fast(recip[:], den[:])
                    bc = small.tile([64, 512], F32, tag="bc", bufs=2,
                                    name="bc")
                    nc.gpsimd.partition_broadcast(bc[:], recip[:])
                    nc.vector.tensor_mul(
                        ao2[g // 2][(g % 2) * 64:(g % 2) * 64 + 64,
                                    qtile * 512:(qtile + 1) * 512],
                        av[0:64, :], bc[:])

                for i in range(n + 1):
                    if i < n:
                        emit_scores_exp(i)
                        for f in pinned.get(i, ()) if isinstance(pinned, dict) else ():
                            f()
                    if i > 0:
                        emit_av(i - 1)
                        want = i * n_f // n
                        while consumed[0] < want:
                            fillers[consumed[0]]()
                            consumed[0] += 1
                while consumed[0] < n_f:
                    fillers[consumed[0]]()
                    consumed[0] += 1

            def oproj_tail(tiles, b, t2s_list):
                _, _, _, _, ao2 = tiles
                for t2 in t2s_list:
                    t2s = slice(t2 * 128, (t2 + 1) * 128)
                    o_sb = epool.tile([128, 4, 512], BF16, tag="osb", bufs=3,
                                      name="osb")
                    for half in range(2):
                        po = pps.tile([128, 2, 512], F32, tag="big2", bufs=2,
                                      name="po2")
                        for dt in range(2):
                            for nt in range(2):
                                nc.tensor.matmul(
                                    po[:, nt, :], ao2[dt][:, t2s],
                                    wo_sb[:, dt, half * 2 + nt, :],
                                    start=dt == 0, stop=dt == 1)
                        if half == 0:
                            nc.scalar.copy(o_sb[:, 0:2, :], po[:])
                        else:
                            nc.vector.tensor_copy(o_sb[:, 2:4, :], po[:])
                    eng = (nc.sync, nc.gpsimd, nc.scalar)[t2 % 3]
                    eng.dma_start(out_p.ap()[b, t2s, :], o_sb[:])

            # ================= schedule =================
            tiles0 = batch_tiles()
            # minimal b0 prefix: K/V tile 0 + first transpose pair + Q for
            # heads 0/1 of query-half 0; everything else is pinned filler
            # work with per-step deadlines derived from its consumers
            nc.vector.memset(tiles0[3][:, :, 64:65], 1.0)  # v1 ones column
            make_pass(tiles0, xcs0, 0, 2)[0]()
            transpose_pair(tiles0, 0)
            make_pass(tiles0, xcs0, 0, 0)[0]()
            make_pass(tiles0, xcs0, 1, 0)[0]()

            tiles1 = batch_tiles()
            nc.vector.memset(tiles1[3][:, :, 64:65], 1.0)

            kv1 = make_pass(tiles0, xcs0, 1, 2, split=CT // 2)
            kv2 = make_pass(tiles0, xcs0, 2, 2, split=CT // 2)
            kv3 = make_pass(tiles0, xcs0, 3, 2, split=CT // 2)
            qb0 = make_pass(tiles0, xcs0, 0, 1, split=CT // 2)
            qb1 = make_pass(tiles0, xcs0, 1, 1, split=CT // 2)
            tp = lambda m: (lambda: transpose_pair(tiles0, m))
            pinned0 = {
                0: [tp(2), kv1[0]], 1: [kv1[1]],
                2: [tp(4)], 3: [tp(6)],
                4: [kv2[0]], 5: [kv2[1]],
                6: [tp(8)], 7: [tp(10)],
                8: [kv3[0]], 9: [kv3[1]],
                10: [tp(12)], 11: [tp(14)],
                12: [qb0[0]], 13: [qb0[1]],
                14: [qb1[0]], 15: [qb1[1]],
            }
            # paced fillers: b0 q-projections for query-half 1, then b1 proj
            fillers0 = []
            for tt in (2, 3):
                for dt in range(2):
                    fillers0.extend(make_pass(tiles0, xcs0, tt, dt,
                                              split=CT // 2))
            fillers0 += b1_proj_fillers(tiles1, 1)
            attention(0, tiles0, fillers0, pinned=pinned0)

            # attention b1: b0 O-proj + first half of b1 O-proj as fillers
            fillers1 = oproj_fillers(tiles0, 0, list(range(16)))
            fillers1 += oproj_fillers(tiles1, 1, list(range(8)))
            attention(1, tiles1, fillers1)
            oproj_tail(tiles1, 1, list(range(8, 16)))

    nc.compile()
    return nc


def _get_nc():
    if "nc" not in _CACHE:
        _CACHE["nc"] = _build()
    return _CACHE["nc"]


def kernel(x, Wq, Wk, Wv, Wo, _trace=False):
    nc = _get_nc()
    bf = ml_dtypes.bfloat16
    xT = np.ascontiguousarray(
        np.asarray(x, np.float32).transpose(2, 0, 1).reshape(DIM, TOKS)
    ).astype(bf)
    Wq = np.asarray(Wq, np.float32)
    Wk = np.asarray(Wk, np.float32)
    Wv = np.asarray(Wv, np.float32)
    Wo = np.asarray(Wo, np.float32)

    in_maps = []
    for c in range(NCORES):
        wq_c = Wq[:, c * DQ:(c + 1) * DQ].astype(bf)
        wkv_c = np.concatenate(
            [Wk[:, c * HD:(c + 1) * HD], Wv[:, c * HD:(c + 1) * HD]], axis=1
        ).astype(bf)
        wo_c = Wo[c * DQ:(c + 1) * DQ, :].astype(bf)
        in_maps.append({"xT": xT, "wq": np.ascontiguousarray(wq_c),
                        "wkv": np.ascontiguousarray(wkv_c),
                        "wo": np.ascontiguousarray(wo_c)})

    res = bass_utils.run_bass_kernel_spmd(
        nc, in_maps, core_ids=list(range(NCORES)), trace=_trace
    )
    out = res.results[0]["out_p"].astype(np.float64)
    for c in range(1, NCORES):
        out += res.results[c]["out_p"].astype(np.float64)
    if _trace:
        kernel.last_exec_time_ns = res.exec_time_ns
        kernel.last_results = res
    return out.astype(np.float32)


kernel.last_exec_time_ns = None
